# revision 6
# baseline (speedup 1.0000x reference)
"""GAT (2-layer, residual, classifier) on 8 Trainium2 NeuronCores.

Strategy (graph/data parallel, per the sharding hint):
 - Nodes sharded by range across 8 cores; each core owns the edges whose
   destination falls in its range (segment softmax + aggregation are
   dst-local).
 - Node-phase matmuls run feature-major (features on partitions).
 - Edge phase uses a degree-bucketed ELL layout: per 128-node group, each
   node's incident edges occupy K slots along the free dimension; source-node
   feature rows (h | alpha_src packed into 512B rows) are fetched with
   dma_gather, softmax coefficients and the weighted aggregation run on the
   vector engine as per-partition ops, so no scatter is ever needed.
 - dma_gather indices are int16, so the 50002-row feature table is addressed
   through two overlapping 32768-row windows (A = rows 0..32767,
   B = rows 17234..50001); every edge is assigned a window and each group's
   slot columns are split into an A-run and a B-run. Padding slots point at
   dummy rows whose alpha_src is -60000 -> exp(logit) == 0, so they
   contribute nothing.
 - Three launches: A (node phase 1), B (edge phase 1 + node phase 2),
   C (edge phase 2 + residual + classifier). Between launches the host only
   reshapes/transposes/casts device-produced tensors into gather tables.
"""

import numpy as np
from contextlib import ExitStack

import concourse.bass as bass
import concourse.mybir as mybir
import concourse.tile as tile
import concourse.bacc as bacc
from concourse import bass_utils

# problem shape (hardcoded per contest contract)
N = 50000
E = 800000
IN_C = 128
HID = 32
HEADS = 4
H1F = HEADS * HID  # 128
OUT_C = 64
NEG = 0.2
NCORES = 8
SH = N // NCORES          # 6250 nodes per core
NG = (SH + 127) // 128    # 49 groups of 128 node-slots
SHP = NG * 128            # 6272 padded node slots

NROWS = N + 2             # dummyA, nodes, dummyB
WIN = 32768
OFF_B = NROWS - WIN       # 17234
A_MAX_SRC = WIN - 2       # 32766: last src reachable via window A (row=src+1)
B_MIN_SRC = OFF_B - 1     # 17233: first src reachable via window B
DUMMY_A = 0
DUMMY_B = WIN - 1         # 32767

TROW1 = 256               # fp16 elems per table-1 row (512B)
TROW2 = 128               # fp32 elems per table-2 row (512B)

F16 = mybir.dt.float16
F32 = mybir.dt.float32
I16 = mybir.dt.int16
OP = mybir.AluOpType
ACT = mybir.ActivationFunctionType


# ---------------------------------------------------------------- host plan

def _wrap_idx(val):
    """[128, K] int16 slot values -> dma_gather wrapped index layout
    [128, 8*K] (element i of the flat gather order at [i%16, i//16],
    replicated to 128 partitions)."""
    p, k = val.shape
    assert p == 128
    w = val.reshape(8, 16, k).transpose(1, 2, 0).reshape(16, 8 * k)
    return np.tile(w, (8, 1))


def _plan(edge_index):
    src = np.concatenate([edge_index[0], np.arange(N, dtype=np.int64)])
    dst = np.concatenate([edge_index[1], np.arange(N, dtype=np.int64)])
    cores = []
    for c in range(NCORES):
        lo = c * SH
        m = (dst >= lo) & (dst < lo + SH)
        s = src[m].astype(np.int64)
        d = (dst[m] - lo).astype(np.int64)
        o = np.argsort(d, kind="stable")
        s, d = s[o], d[o]
        deg = np.bincount(d, minlength=SH)
        amust = np.bincount(d[s <= B_MIN_SRC - 1], minlength=SH)
        bmust = np.bincount(d[s >= A_MAX_SRC + 1], minlength=SH)
        perm = np.lexsort((-amust, -deg))
        starts = np.concatenate([[0], np.cumsum(deg)])
        cores.append(dict(s=s, deg=deg, a=amust, b=bmust, perm=perm, starts=starts))

    KA = np.zeros(NG, np.int64)
    KB = np.zeros(NG, np.int64)
    for g in range(NG):
        for p in cores:
            nodes = p["perm"][g * 128:(g + 1) * 128]
            if len(nodes):
                KA[g] = max(KA[g], p["a"][nodes].max())
    for g in range(NG):
        for p in cores:
            nodes = p["perm"][g * 128:(g + 1) * 128]
            if len(nodes):
                KB[g] = max(KB[g], p["b"][nodes].max(),
                            p["deg"][nodes].max() - KA[g])
    KA = np.maximum(KA, 1)
    KB = np.maximum(KB, 1)

    idx_maps = []
    for p in cores:
        blocks = []
        for g in range(NG):
            nodes = p["perm"][g * 128:(g + 1) * 128]
            vA = np.full((128, KA[g]), DUMMY_A, np.int16)
            vB = np.full((128, KB[g]), DUMMY_B, np.int16)
            for pi, n in enumerate(nodes):
                es = p["s"][p["starts"][n]:p["starts"][n + 1]]
                sa = es[es <= B_MIN_SRC - 1]
                sb = es[es >= A_MAX_SRC + 1]
                fl = es[(es > B_MIN_SRC - 1) & (es < A_MAX_SRC + 1)]
                a_load = max(len(sa), len(es) - KB[g])
                take = a_load - len(sa)
                av = np.concatenate([sa, fl[:take]]) + 1
                bv = np.concatenate([fl[take:], sb]) + 1 - OFF_B
                assert len(av) <= KA[g] and len(bv) <= KB[g]
                vA[pi, :len(av)] = av.astype(np.int16)
                vB[pi, :len(bv)] = bv.astype(np.int16)
            blocks.append(_wrap_idx(vA))
            blocks.append(_wrap_idx(vB))
        idx_maps.append(np.ascontiguousarray(np.concatenate(blocks, axis=1)))

    return cores, KA, KB, idx_maps


# ------------------------------------------------------------ launch builders

def _chunks(total, step=512):
    return [(o, min(step, total - o)) for o in range(0, total, step)]


def _elu(nc, pool, v, cols, out_ap):
    """out = elu(v) for an f-major fp32 SBUF tile v [P, cols]; out_ap may be
    a different dtype (cast on the final op)."""
    r = pool.tile(list(v.shape), F32, tag="elu_r")
    m = pool.tile(list(v.shape), F32, tag="elu_m")
    nc.vector.tensor_scalar_max(r[:], v[:], 0.0)
    nc.vector.tensor_scalar_min(m[:], v[:], 0.0)
    nc.scalar.activation(m[:], m[:], ACT.Exp)
    nc.vector.tensor_tensor(r[:], r[:], m[:], OP.add)
    nc.vector.tensor_scalar_add(out_ap, r[:], -1.0)


def build_launch_a(nc):
    xT = nc.dram_tensor("xT", [IN_C, SH], F16, kind="ExternalInput").ap()
    W1 = nc.dram_tensor("W1f", [IN_C, H1F], F16, kind="ExternalInput").ap()
    Ms1 = nc.dram_tensor("Ms1", [H1F, HEADS], F16, kind="ExternalInput").ap()
    Md1 = nc.dram_tensor("Md1", [H1F, HEADS], F16, kind="ExternalInput").ap()
    Wres = nc.dram_tensor("Wresf", [IN_C, OUT_C], F16, kind="ExternalInput").ap()
    bres = nc.dram_tensor("bres", [OUT_C, 1], F32, kind="ExternalInput").ap()
    h1t = nc.dram_tensor("h1t", [H1F, SH], F16, kind="ExternalOutput").ap()
    as1t = nc.dram_tensor("as1t", [HEADS, SH], F32, kind="ExternalOutput").ap()
    ad1t = nc.dram_tensor("ad1t", [HEADS, SH], F32, kind="ExternalOutput").ap()
    xrest = nc.dram_tensor("xrest", [OUT_C, SH], F32, kind="ExternalOutput").ap()

    with tile.TileContext(nc) as tc:
        with tc.tile_pool(name="sb", bufs=1) as pool, \
             tc.tile_pool(name="ps", bufs=2, space="PSUM") as pps:
            x_sb = pool.tile([IN_C, SH], F16)
            w1_sb = pool.tile([IN_C, H1F], F16)
            ms_sb = pool.tile([H1F, HEADS], F16)
            md_sb = pool.tile([H1F, HEADS], F16)
            wr_sb = pool.tile([IN_C, OUT_C], F16)
            br_sb = pool.tile([OUT_C, 1], F32)
            h1_sb = pool.tile([H1F, SH], F16)
            as_sb = pool.tile([HEADS, SH], F32)
            ad_sb = pool.tile([HEADS, SH], F32)
            xr_sb = pool.tile([OUT_C, SH], F32)
            nc.sync.dma_start(x_sb[:], xT)
            nc.sync.dma_start(w1_sb[:], W1)
            nc.sync.dma_start(ms_sb[:], Ms1)
            nc.sync.dma_start(md_sb[:], Md1)
            nc.sync.dma_start(wr_sb[:], Wres)
            nc.sync.dma_start(br_sb[:], bres)
            for o, n in _chunks(SH):
                ph = pps.tile([H1F, 512], F32, tag="ph")
                nc.tensor.matmul(ph[:, :n], w1_sb[:], x_sb[:, o:o + n])
                nc.vector.tensor_copy(h1_sb[:, o:o + n], ph[:, :n])
                pr = pps.tile([OUT_C, 512], F32, tag="pr")
                nc.tensor.matmul(pr[:, :n], wr_sb[:], x_sb[:, o:o + n])
                nc.scalar.activation(xr_sb[:, o:o + n], pr[:, :n], ACT.Identity,
                                     bias=br_sb[:])
            for o, n in _chunks(SH):
                pa = pps.tile([HEADS, 512], F32, tag="pa")
                nc.tensor.matmul(pa[:, :n], ms_sb[:], h1_sb[:, o:o + n])
                nc.vector.tensor_copy(as_sb[:, o:o + n], pa[:, :n])
                pd = pps.tile([HEADS, 512], F32, tag="pd")
                nc.tensor.matmul(pd[:, :n], md_sb[:], h1_sb[:, o:o + n])
                nc.vector.tensor_copy(ad_sb[:, o:o + n], pd[:, :n])
            nc.sync.dma_start(h1t, h1_sb[:])
            nc.sync.dma_start(as1t, as_sb[:])
            nc.sync.dma_start(ad1t, ad_sb[:])
            nc.sync.dma_start(xrest, xr_sb[:])
    nc.compile()
    return nc


def _edge_phase(nc, tc, pool, pps, gwinA, gwinB, idx_sb, adP_sb, KA, KB,
                heads, feat, trow, gdt, out_group_cb):
    """Shared edge phase: for each group, gather + segment softmax + weighted
    aggregation; calls out_group_cb(g, o_sb) with the node-major [128, feat]
    fp32 aggregate."""
    kmax = int(max(KA[g] + KB[g] for g in range(NG)))
    hc = feat // heads
    icol = 0
    for g in range(NG):
        ka, kb, kg = int(KA[g]), int(KB[g]), int(KA[g] + KB[g])
        gt = pool.tile([128, kmax, trow], gdt, tag="gt")
        nc.gpsimd.dma_gather(
            gt[:][:, 0:ka, :], gwinA, idx_sb[:][:, icol:icol + 8 * ka],
            128 * ka, 128 * ka, trow, single_packet=False)
        icol += 8 * ka
        nc.gpsimd.dma_gather(
            gt[:][:, ka:kg, :], gwinB, idx_sb[:][:, icol:icol + 8 * kb],
            128 * kb, 128 * kb, trow, single_packet=False)
        icol += 8 * kb

        lg = pool.tile([128, heads, kmax], F32, tag="lg")
        lt = pool.tile([128, heads, kmax], F32, tag="lt")
        ex = pool.tile([128, heads, kmax], F16, tag="ex")
        for h in range(heads):
            nc.vector.tensor_scalar(
                lg[:][:, h, 0:kg], gt[:][:, 0:kg, feat + h],
                adP_sb[:][:, g, h:h + 1], None, OP.add)
        # leaky_relu then exp
        nc.vector.tensor_scalar_mul(lt[:][:, :, 0:kg], lg[:][:, :, 0:kg], NEG)
        nc.vector.tensor_tensor(lg[:][:, :, 0:kg], lg[:][:, :, 0:kg],
                                lt[:][:, :, 0:kg], OP.max)
        nc.scalar.activation(ex[:][:, :, 0:kg], lg[:][:, :, 0:kg], ACT.Exp)

        z = pool.tile([128, heads], F32, tag="z")
        zr = pool.tile([128, heads], F32, tag="zr")
        nc.vector.tensor_reduce(z[:], ex[:][:, :, 0:kg], mybir.AxisListType.X,
                                OP.add)
        nc.vector.reciprocal(zr[:], z[:])
        for h in range(heads):
            nc.vector.tensor_tensor(
                gt[:][:, 0:kg, h * hc:(h + 1) * hc],
                gt[:][:, 0:kg, h * hc:(h + 1) * hc],
                ex[:][:, h:h + 1, 0:kg].rearrange("p o k -> p k o")
                    .broadcast_to([128, kg, hc]),
                OP.mult)
        o_sb = pool.tile([128, feat], F32, tag="o")
        nc.vector.tensor_reduce(
            o_sb[:], gt[:].rearrange("p k c -> p c k")[:, 0:feat, 0:kg],
            mybir.AxisListType.X, OP.add)
        for h in range(heads):
            nc.vector.tensor_scalar_mul(
                o_sb[:][:, h * hc:(h + 1) * hc],
                o_sb[:][:, h * hc:(h + 1) * hc], zr[:][:, h:h + 1])
        out_group_cb(g, o_sb)


def build_launch_b(nc, KA, KB, ncols):
    tab = nc.dram_tensor("tab1", [NROWS, TROW1], F16, kind="ExternalInput").ap()
    idx = nc.dram_tensor("idx1", [128, ncols], I16, kind="ExternalInput").ap()
    adP = nc.dram_tensor("adP1", [128, NG, HEADS], F32, kind="ExternalInput").ap()
    b1c = nc.dram_tensor("b1c", [H1F, 1], F32, kind="ExternalInput").ap()
    W2 = nc.dram_tensor("W2f", [H1F, OUT_C], F16, kind="ExternalInput").ap()
    Ms2 = nc.dram_tensor("Ms2", [OUT_C, 1], F16, kind="ExternalInput").ap()
    Md2 = nc.dram_tensor("Md2", [OUT_C, 1], F16, kind="ExternalInput").ap()
    idm = nc.dram_tensor("idm", [128, 128], F32, kind="ExternalInput").ap()
    h2t = nc.dram_tensor("h2t", [OUT_C, SHP], F16, kind="ExternalOutput").ap()
    as2t = nc.dram_tensor("as2t", [1, SHP], F32, kind="ExternalOutput").ap()
    ad2t = nc.dram_tensor("ad2t", [1, SHP], F32, kind="ExternalOutput").ap()

    with tile.TileContext(nc) as tc:
        with tc.tile_pool(name="st", bufs=1) as spool, \
             tc.tile_pool(name="gp", bufs=3) as gpool, \
             tc.tile_pool(name="ps", bufs=2, space="PSUM") as pps:
            idx_sb = spool.tile([128, ncols], I16)
            adP_sb = spool.tile([128, NG, HEADS], F32)
            b1_sb = spool.tile([H1F, 1], F32)
            w2_sb = spool.tile([H1F, OUT_C], F16)
            ms_sb = spool.tile([OUT_C, 1], F16)
            md_sb = spool.tile([OUT_C, 1], F16)
            id_sb = spool.tile([128, 128], F32)
            elu_sb = spool.tile([H1F, SHP], F16)
            h2_sb = spool.tile([OUT_C, SHP], F16)
            as2_sb = spool.tile([1, SHP], F32)
            ad2_sb = spool.tile([1, SHP], F32)
            nc.sync.dma_start(idx_sb[:], idx)
            nc.sync.dma_start(adP_sb[:], adP)
            nc.sync.dma_start(b1_sb[:], b1c)
            nc.sync.dma_start(w2_sb[:], W2)
            nc.sync.dma_start(ms_sb[:], Ms2)
            nc.sync.dma_start(md_sb[:], Md2)
            nc.sync.dma_start(id_sb[:], idm)

            def finish_group(g, o_sb):
                pt = pps.tile([128, 128], F32, tag="pt")
                nc.tensor.transpose(pt[:], o_sb[:], id_sb[:])
                v = gpool.tile([128, 128], F32, tag="v")
                nc.scalar.activation(v[:], pt[:], ACT.Identity, bias=b1_sb[:])
                _elu(nc, gpool, v, 128, elu_sb[:][:, g * 128:(g + 1) * 128])

            _edge_phase(nc, tc, gpool, pps, tab[0:WIN, :], tab[OFF_B:NROWS, :],
                        idx_sb, adP_sb, KA, KB, HEADS, H1F, TROW1, F16,
                        finish_group)

            for o, n in _chunks(SHP):
                p2 = pps.tile([OUT_C, 512], F32, tag="p2")
                nc.tensor.matmul(p2[:, :n], w2_sb[:], elu_sb[:][:, o:o + n])
                nc.vector.tensor_copy(h2_sb[:][:, o:o + n], p2[:, :n])
            for o, n in _chunks(SHP):
                pa = pps.tile([1, 512], F32, tag="pa2")
                nc.tensor.matmul(pa[:, :n], ms_sb[:], h2_sb[:][:, o:o + n])
                nc.vector.tensor_copy(as2_sb[:][:, o:o + n], pa[:, :n])
                pd = pps.tile([1, 512], F32, tag="pd2")
                nc.tensor.matmul(pd[:, :n], md_sb[:], h2_sb[:][:, o:o + n])
                nc.vector.tensor_copy(ad2_sb[:][:, o:o + n], pd[:, :n])
            nc.sync.dma_start(h2t, h2_sb[:])
            nc.sync.dma_start(as2t, as2_sb[:])
            nc.sync.dma_start(ad2t, ad2_sb[:])
    nc.compile()
    return nc


def build_launch_c(nc, KA, KB, ncols):
    tab = nc.dram_tensor("tab2", [NROWS, TROW2], F32, kind="ExternalInput").ap()
    idx = nc.dram_tensor("idx2", [128, ncols], I16, kind="ExternalInput").ap()
    adP = nc.dram_tensor("adP2", [128, NG, 1], F32, kind="ExternalInput").ap()
    b2c = nc.dram_tensor("b2c", [OUT_C, 1], F32, kind="ExternalInput").ap()
    xres = nc.dram_tensor("xresP", [OUT_C, SHP], F32, kind="ExternalInput").ap()
    Wc1 = nc.dram_tensor("Wc1f", [OUT_C, 64], F16, kind="ExternalInput").ap()
    bc1 = nc.dram_tensor("bc1c", [64, 1], F32, kind="ExternalInput").ap()
    Wc2 = nc.dram_tensor("Wc2f", [64, 2], F16, kind="ExternalInput").ap()
    bc2 = nc.dram_tensor("bc2c", [2, 1], F32, kind="ExternalInput").ap()
    idm = nc.dram_tensor("idm", [128, 128], F32, kind="ExternalInput").ap()
    yt = nc.dram_tensor("yt", [2, SHP], F32, kind="ExternalOutput").ap()

    with tile.TileContext(nc) as tc:
        with tc.tile_pool(name="st", bufs=1) as spool, \
             tc.tile_pool(name="gp", bufs=3) as gpool, \
             tc.tile_pool(name="ps", bufs=2, space="PSUM") as pps:
            idx_sb = spool.tile([128, ncols], I16)
            adP_sb = spool.tile([128, NG, 1], F32)
            b2_sb = spool.tile([OUT_C, 1], F32)
            xr_sb = spool.tile([OUT_C, SHP], F32)
            w1_sb = spool.tile([OUT_C, 64], F16)
            b1_sb = spool.tile([64, 1], F32)
            w2_sb = spool.tile([64, 2], F16)
            b2c_sb = spool.tile([2, 1], F32)
            id_sb = spool.tile([128, 128], F32)
            y0_sb = spool.tile([OUT_C, SHP], F16)
            y1_sb = spool.tile([64, SHP], F16)
            y_sb = spool.tile([2, SHP], F32)
            nc.sync.dma_start(idx_sb[:], idx)
            nc.sync.dma_start(adP_sb[:], adP)
            nc.sync.dma_start(b2_sb[:], b2c)
            nc.sync.dma_start(xr_sb[:], xres)
            nc.sync.dma_start(w1_sb[:], Wc1)
            nc.sync.dma_start(b1_sb[:], bc1)
            nc.sync.dma_start(w2_sb[:], Wc2)
            nc.sync.dma_start(b2c_sb[:], bc2)
            nc.sync.dma_start(id_sb[:], idm)

            def finish_group(g, o_sb):
                pt = pps.tile([OUT_C, 128], F32, tag="pt")
                nc.tensor.transpose(pt[:], o_sb[:][:, 0:OUT_C], id_sb[:])
                v = gpool.tile([OUT_C, 128], F32, tag="v")
                nc.scalar.activation(v[:], pt[:], ACT.Identity, bias=b2_sb[:])
                e = gpool.tile([OUT_C, 128], F32, tag="e")
                _elu(nc, gpool, v, 128, e[:])
                nc.vector.tensor_tensor(
                    y0_sb[:][:, g * 128:(g + 1) * 128], e[:],
                    xr_sb[:][:, g * 128:(g + 1) * 128], OP.add)

            _edge_phase(nc, tc, gpool, pps, tab[0:WIN, :], tab[OFF_B:NROWS, :],
                        idx_sb, adP_sb, KA, KB, 1, OUT_C, TROW2, F32,
                        finish_group)

            for o, n in _chunks(SHP):
                p1 = pps.tile([64, 512], F32, tag="p1")
                nc.tensor.matmul(p1[:, :n], w1_sb[:], y0_sb[:][:, o:o + n])
                nc.scalar.activation(y1_sb[:][:, o:o + n], p1[:, :n], ACT.Relu,
                                     bias=b1_sb[:])
                p2 = pps.tile([2, 512], F32, tag="p2")
                nc.tensor.matmul(p2[:, :n], w2_sb[:], y1_sb[:][:, o:o + n])
                nc.scalar.activation(y_sb[:][:, o:o + n], p2[:, :n],
                                     ACT.Identity, bias=b2c_sb[:])
            nc.sync.dma_start(yt, y_sb[:])
    nc.compile()
    return nc


# ------------------------------------------------------------------- kernel

_LAST_RUNS = []


def _run(nc, in_maps, name=""):
    _LAST_RUNS.append((name, nc, in_maps))
    return bass_utils.run_bass_kernel_spmd(nc, in_maps,
                                           core_ids=list(range(NCORES)))


def _ms_mat(a, heads, hid):
    m = np.zeros((heads * hid, heads), np.float32)
    for h in range(heads):
        m[h * hid:(h + 1) * hid, h] = a[h]
    return m


_CACHE = {}


def _get_programs(edge_index):
    key = edge_index.tobytes()[:64] + str(edge_index.sum()).encode()
    if key not in _CACHE:
        cores, KA, KB, idx_maps = _plan(edge_index)
        ncols = int(8 * (KA.sum() + KB.sum()))
        nca = build_launch_a(bacc.Bacc("TRN2", target_bir_lowering=False,
                                       debug=False, num_devices=NCORES))
        ncb = build_launch_b(bacc.Bacc("TRN2", target_bir_lowering=False,
                                       debug=False, num_devices=NCORES),
                             KA, KB, ncols)
        ncc = build_launch_c(bacc.Bacc("TRN2", target_bir_lowering=False,
                                       debug=False, num_devices=NCORES),
                             KA, KB, ncols)
        _CACHE[key] = (cores, KA, KB, idx_maps, ncols, nca, ncb, ncc)
    return _CACHE[key]


def kernel(x, edge_index, W1, a_src1, a_dst1, b1, W2, a_src2, a_dst2, b2,
           Wres, bres, Wc1, bc1, Wc2, bc2):
    x = np.asarray(x, np.float32)
    edge_index = np.asarray(edge_index, np.int32)
    cores, KA, KB, idx_maps, ncols, nca, ncb, ncc = _get_programs(edge_index)

    idm = np.eye(128, dtype=np.float32)

    # ---- launch A: node phase 1 (h1 = x@W1, alpha_s/d, residual)
    W1f = np.asarray(W1, np.float16)
    Ms1 = _ms_mat(np.asarray(a_src1), HEADS, HID).astype(np.float16)
    Md1 = _ms_mat(np.asarray(a_dst1), HEADS, HID).astype(np.float16)
    Wresf = np.asarray(Wres, np.float16)
    bresc = np.asarray(bres, np.float32).reshape(OUT_C, 1)
    in_a = []
    for c in range(NCORES):
        xT = np.ascontiguousarray(x[c * SH:(c + 1) * SH].T.astype(np.float16))
        in_a.append(dict(xT=xT, W1f=W1f, Ms1=Ms1, Md1=Md1, Wresf=Wresf,
                         bres=bresc))
    _LAST_RUNS.clear()
    res_a = _run(nca, in_a, 'A')

    h1 = np.concatenate([res_a.results[c]["h1t"].T for c in range(NCORES)], 0)
    as1 = np.concatenate([res_a.results[c]["as1t"].T for c in range(NCORES)], 0)
    ad1 = np.concatenate([res_a.results[c]["ad1t"].T for c in range(NCORES)], 0)
    xresT = [res_a.results[c]["xrest"] for c in range(NCORES)]

    # ---- host: pack gather table 1 (fp16, 512B rows) + permuted ad columns
    tab1 = np.zeros((NROWS, TROW1), np.float16)
    tab1[1:N + 1, :H1F] = h1.astype(np.float16)
    tab1[1:N + 1, H1F:H1F + HEADS] = as1.astype(np.float16)
    tab1[0, H1F:H1F + HEADS] = -60000.0
    tab1[N + 1, H1F:H1F + HEADS] = -60000.0

    b1c = np.asarray(b1, np.float32).reshape(H1F, 1)
    W2f = np.asarray(W2, np.float16)
    Ms2 = _ms_mat(np.asarray(a_src2), 1, OUT_C).astype(np.float16)
    Md2 = _ms_mat(np.asarray(a_dst2), 1, OUT_C).astype(np.float16)
    in_b = []
    for c in range(NCORES):
        perm = cores[c]["perm"]
        adp = np.zeros((SHP, HEADS), np.float32)
        adp[:SH] = ad1[c * SH + perm]
        adp = adp.reshape(NG, 128, HEADS).transpose(1, 0, 2)
        in_b.append(dict(tab1=tab1, idx1=idx_maps[c],
                         adP1=np.ascontiguousarray(adp), b1c=b1c, W2f=W2f,
                         Ms2=Ms2, Md2=Md2, idm=idm))
    res_b = _run(ncb, in_b, 'B')

    # ---- host: pack gather table 2 (fp32, 512B rows); un-permute h2/as2/ad2
    h2 = np.zeros((N, OUT_C), np.float32)
    as2 = np.zeros(N, np.float32)
    ad2 = np.zeros(N, np.float32)
    for c in range(NCORES):
        perm = cores[c]["perm"]
        h2[c * SH + perm] = res_b.results[c]["h2t"].T[:SH].astype(np.float32)
        as2[c * SH + perm] = res_b.results[c]["as2t"][0, :SH]
        ad2[c * SH + perm] = res_b.results[c]["ad2t"][0, :SH]
    tab2 = np.zeros((NROWS, TROW2), np.float32)
    tab2[1:N + 1, :OUT_C] = h2
    tab2[1:N + 1, OUT_C] = as2
    tab2[0, OUT_C] = -1e30
    tab2[N + 1, OUT_C] = -1e30

    b2c = np.asarray(b2, np.float32).reshape(OUT_C, 1)
    Wc1f = np.asarray(Wc1, np.float16)
    bc1c = np.asarray(bc1, np.float32).reshape(64, 1)
    Wc2f = np.asarray(Wc2, np.float16)
    bc2c = np.asarray(bc2, np.float32).reshape(2, 1)
    in_c = []
    for c in range(NCORES):
        perm = cores[c]["perm"]
        adp = np.zeros((SHP, 1), np.float32)
        adp[:SH, 0] = ad2[c * SH + perm]
        adp = adp.reshape(NG, 128, 1).transpose(1, 0, 2)
        xrp = np.zeros((OUT_C, SHP), np.float32)
        xrp[:, :SH] = xresT[c][:, perm]
        in_c.append(dict(tab2=tab2, idx2=idx_maps[c],
                         adP2=np.ascontiguousarray(adp), b2c=b2c,
                         xresP=xrp, Wc1f=Wc1f, bc1c=bc1c, Wc2f=Wc2f,
                         bc2c=bc2c, idm=idm))
    res_c = _run(ncc, in_c, 'C')

    out = np.zeros((N, 2), np.float32)
    for c in range(NCORES):
        perm = cores[c]["perm"]
        out[c * SH + perm] = res_c.results[c]["yt"].T[:SH]
    return out


# revision 7
# speedup vs baseline: 99.3830x; 99.3830x over previous
"""GAT (2-layer, residual, classifier) on 8 Trainium2 NeuronCores.

Strategy (graph/data parallel, per the sharding hint):
 - Nodes sharded by range across 8 cores; each core owns the edges whose
   destination falls in its range (segment softmax + aggregation are
   dst-local).
 - Node-phase matmuls run feature-major (features on partitions).
 - Edge phase uses a degree-bucketed ELL layout: per 128-node group, each
   node's incident edges occupy K slots along the free dimension; source-node
   feature rows (h | alpha_src packed into 512B rows) are fetched with
   dma_gather, softmax coefficients and the weighted aggregation run on the
   vector engine as per-partition ops, so no scatter is ever needed.
 - dma_gather indices are int16, so the 50002-row feature table is addressed
   through two overlapping 32768-row windows (A = rows 0..32767,
   B = rows 17234..50001); every edge is assigned a window and each group's
   slot columns are split into an A-run and a B-run. Padding slots point at
   dummy rows whose alpha_src is -60000 -> exp(logit) == 0, so they
   contribute nothing.
 - Three launches: A (node phase 1), B (edge phase 1 + node phase 2),
   C (edge phase 2 + residual + classifier). Between launches the host only
   reshapes/transposes/casts device-produced tensors into gather tables.
"""

import numpy as np
from contextlib import ExitStack

import concourse.bass as bass
import concourse.mybir as mybir
import concourse.tile as tile
import concourse.bacc as bacc
from concourse import bass_utils

# problem shape (hardcoded per contest contract)
N = 50000
E = 800000
IN_C = 128
HID = 32
HEADS = 4
H1F = HEADS * HID  # 128
OUT_C = 64
NEG = 0.2
NCORES = 8
SH = N // NCORES          # 6250 nodes per core
NG = (SH + 127) // 128    # 49 groups of 128 node-slots
SHP = NG * 128            # 6272 padded node slots

NROWS = N + 2             # dummyA, nodes, dummyB
WIN = 32768
OFF_B = NROWS - WIN       # 17234
A_MAX_SRC = WIN - 2       # 32766: last src reachable via window A (row=src+1)
B_MIN_SRC = OFF_B - 1     # 17233: first src reachable via window B
DUMMY_A = 0
DUMMY_B = WIN - 1         # 32767

TROW1 = 256               # fp16 elems per table-1 row (512B)
TROW2 = 128               # fp32 elems per table-2 row (512B)

F16 = mybir.dt.float16
F32 = mybir.dt.float32
I16 = mybir.dt.int16
OP = mybir.AluOpType
ACT = mybir.ActivationFunctionType


# ---------------------------------------------------------------- host plan

def _wrap_idx(val):
    """[128, K] int16 slot values -> dma_gather wrapped index layout
    [128, 8*K] (element i of the flat gather order at [i%16, i//16],
    replicated to 128 partitions)."""
    p, k = val.shape
    assert p == 128
    w = val.reshape(8, 16, k).transpose(1, 2, 0).reshape(16, 8 * k)
    return np.tile(w, (8, 1))


def _plan(edge_index):
    src = np.concatenate([edge_index[0], np.arange(N, dtype=np.int64)])
    dst = np.concatenate([edge_index[1], np.arange(N, dtype=np.int64)])
    cores = []
    for c in range(NCORES):
        lo = c * SH
        m = (dst >= lo) & (dst < lo + SH)
        s = src[m].astype(np.int64)
        d = (dst[m] - lo).astype(np.int64)
        o = np.argsort(d, kind="stable")
        s, d = s[o], d[o]
        deg = np.bincount(d, minlength=SH)
        amust = np.bincount(d[s <= B_MIN_SRC - 1], minlength=SH)
        bmust = np.bincount(d[s >= A_MAX_SRC + 1], minlength=SH)
        perm = np.lexsort((-amust, -deg))
        starts = np.concatenate([[0], np.cumsum(deg)])
        cores.append(dict(s=s, deg=deg, a=amust, b=bmust, perm=perm, starts=starts))

    KA = np.zeros(NG, np.int64)
    KB = np.zeros(NG, np.int64)
    for g in range(NG):
        for p in cores:
            nodes = p["perm"][g * 128:(g + 1) * 128]
            if len(nodes):
                KA[g] = max(KA[g], p["a"][nodes].max())
    for g in range(NG):
        for p in cores:
            nodes = p["perm"][g * 128:(g + 1) * 128]
            if len(nodes):
                KB[g] = max(KB[g], p["b"][nodes].max(),
                            p["deg"][nodes].max() - KA[g])
    KA = np.maximum(KA, 1)
    KB = np.maximum(KB, 1)

    idx_maps = []
    for p in cores:
        blocks = []
        for g in range(NG):
            nodes = p["perm"][g * 128:(g + 1) * 128]
            vA = np.full((128, KA[g]), DUMMY_A, np.int16)
            vB = np.full((128, KB[g]), DUMMY_B, np.int16)
            for pi, n in enumerate(nodes):
                es = p["s"][p["starts"][n]:p["starts"][n + 1]]
                sa = es[es <= B_MIN_SRC - 1]
                sb = es[es >= A_MAX_SRC + 1]
                fl = es[(es > B_MIN_SRC - 1) & (es < A_MAX_SRC + 1)]
                a_load = max(len(sa), len(es) - KB[g])
                take = a_load - len(sa)
                av = np.concatenate([sa, fl[:take]]) + 1
                bv = np.concatenate([fl[take:], sb]) + 1 - OFF_B
                assert len(av) <= KA[g] and len(bv) <= KB[g]
                vA[pi, :len(av)] = av.astype(np.int16)
                vB[pi, :len(bv)] = bv.astype(np.int16)
            blocks.append(_wrap_idx(vA))
            blocks.append(_wrap_idx(vB))
        idx_maps.append(np.ascontiguousarray(np.concatenate(blocks, axis=1)))

    return cores, KA, KB, idx_maps


# ------------------------------------------------------------ launch builders

def _chunks(total, step=512):
    return [(o, min(step, total - o)) for o in range(0, total, step)]


def _elu(nc, pool, v, cols, out_ap):
    """out = elu(v) for an f-major fp32 SBUF tile v [P, cols]; out_ap may be
    a different dtype (cast on the final op)."""
    r = pool.tile(list(v.shape), F32, tag="elu_r")
    m = pool.tile(list(v.shape), F32, tag="elu_m")
    nc.vector.tensor_scalar_max(r[:], v[:], 0.0)
    nc.vector.tensor_scalar_min(m[:], v[:], 0.0)
    nc.scalar.activation(m[:], m[:], ACT.Exp)
    nc.vector.tensor_tensor(r[:], r[:], m[:], OP.add)
    nc.vector.tensor_scalar_add(out_ap, r[:], -1.0)


def build_launch_a(nc, repeat=0):
    xT = nc.dram_tensor("xT", [IN_C, SH], F16, kind="ExternalInput").ap()
    W1 = nc.dram_tensor("W1f", [IN_C, H1F], F16, kind="ExternalInput").ap()
    Ms1 = nc.dram_tensor("Ms1", [H1F, HEADS], F16, kind="ExternalInput").ap()
    Md1 = nc.dram_tensor("Md1", [H1F, HEADS], F16, kind="ExternalInput").ap()
    Wres = nc.dram_tensor("Wresf", [IN_C, OUT_C], F16, kind="ExternalInput").ap()
    bres = nc.dram_tensor("bres", [OUT_C, 1], F32, kind="ExternalInput").ap()
    h1t = nc.dram_tensor("h1t", [H1F, SH], F16, kind="ExternalOutput").ap()
    as1t = nc.dram_tensor("as1t", [HEADS, SH], F32, kind="ExternalOutput").ap()
    ad1t = nc.dram_tensor("ad1t", [HEADS, SH], F32, kind="ExternalOutput").ap()
    xrest = nc.dram_tensor("xrest", [OUT_C, SH], F32, kind="ExternalOutput").ap()

    with tile.TileContext(nc) as tc:
        with tc.tile_pool(name="sb", bufs=1) as pool, \
             tc.tile_pool(name="ps", bufs=2, space="PSUM") as pps:
            x_sb = pool.tile([IN_C, SH], F16)
            w1_sb = pool.tile([IN_C, H1F], F16)
            ms_sb = pool.tile([H1F, HEADS], F16)
            md_sb = pool.tile([H1F, HEADS], F16)
            wr_sb = pool.tile([IN_C, OUT_C], F16)
            br_sb = pool.tile([OUT_C, 1], F32)
            h1_sb = pool.tile([H1F, SH], F16)
            as_sb = pool.tile([HEADS, SH], F32)
            ad_sb = pool.tile([HEADS, SH], F32)
            xr_sb = pool.tile([OUT_C, SH], F32)
            nc.sync.dma_start(x_sb[:], xT)
            nc.sync.dma_start(w1_sb[:], W1)
            nc.sync.dma_start(ms_sb[:], Ms1)
            nc.sync.dma_start(md_sb[:], Md1)
            nc.sync.dma_start(wr_sb[:], Wres)
            nc.sync.dma_start(br_sb[:], bres)
            rep = ExitStack()
            if repeat:
                rep.enter_context(tc.For_i(0, repeat, 1))
            for o, n in _chunks(SH):
                ph = pps.tile([H1F, 512], F32, tag="ph")
                nc.tensor.matmul(ph[:, :n], w1_sb[:], x_sb[:, o:o + n])
                nc.vector.tensor_copy(h1_sb[:, o:o + n], ph[:, :n])
                pr = pps.tile([OUT_C, 512], F32, tag="pr")
                nc.tensor.matmul(pr[:, :n], wr_sb[:], x_sb[:, o:o + n])
                nc.scalar.activation(xr_sb[:, o:o + n], pr[:, :n], ACT.Identity,
                                     bias=br_sb[:])
            for o, n in _chunks(SH):
                pa = pps.tile([HEADS, 512], F32, tag="pa")
                nc.tensor.matmul(pa[:, :n], ms_sb[:], h1_sb[:, o:o + n])
                nc.vector.tensor_copy(as_sb[:, o:o + n], pa[:, :n])
                pd = pps.tile([HEADS, 512], F32, tag="pd")
                nc.tensor.matmul(pd[:, :n], md_sb[:], h1_sb[:, o:o + n])
                nc.vector.tensor_copy(ad_sb[:, o:o + n], pd[:, :n])
            nc.sync.dma_start(h1t, h1_sb[:])
            nc.sync.dma_start(as1t, as_sb[:])
            nc.sync.dma_start(ad1t, ad_sb[:])
            nc.sync.dma_start(xrest, xr_sb[:])
            rep.close()
    nc.compile()
    return nc


def _edge_phase(nc, tc, pool, pps, gwinA, gwinB, idx_sb, adP_sb, KA, KB,
                heads, feat, trow, gdt, out_group_cb):
    """Shared edge phase: for each group, gather + segment softmax + weighted
    aggregation; calls out_group_cb(g, o_sb) with the node-major [128, feat]
    fp32 aggregate."""
    kmax = int(max(KA[g] + KB[g] for g in range(NG)))
    hc = feat // heads
    icol = 0
    for g in range(NG):
        ka, kb, kg = int(KA[g]), int(KB[g]), int(KA[g] + KB[g])
        gt = pool.tile([128, kmax, trow], gdt, tag="gt")
        nc.gpsimd.dma_gather(
            gt[:][:, 0:ka, :], gwinA, idx_sb[:][:, icol:icol + 8 * ka],
            128 * ka, 128 * ka, trow, single_packet=False)
        icol += 8 * ka
        nc.gpsimd.dma_gather(
            gt[:][:, ka:kg, :], gwinB, idx_sb[:][:, icol:icol + 8 * kb],
            128 * kb, 128 * kb, trow, single_packet=False)
        icol += 8 * kb

        lg = pool.tile([128, heads, kmax], F32, tag="lg")
        lt = pool.tile([128, heads, kmax], F32, tag="lt")
        ex = pool.tile([128, heads, kmax], F16, tag="ex")
        for h in range(heads):
            nc.vector.tensor_scalar(
                lg[:][:, h, 0:kg], gt[:][:, 0:kg, feat + h],
                adP_sb[:][:, g, h:h + 1], None, OP.add)
        # leaky_relu then exp
        nc.vector.tensor_scalar_mul(lt[:][:, :, 0:kg], lg[:][:, :, 0:kg], NEG)
        nc.vector.tensor_tensor(lg[:][:, :, 0:kg], lg[:][:, :, 0:kg],
                                lt[:][:, :, 0:kg], OP.max)
        nc.scalar.activation(ex[:][:, :, 0:kg], lg[:][:, :, 0:kg], ACT.Exp)

        z = pool.tile([128, heads], F32, tag="z")
        zr = pool.tile([128, heads], F32, tag="zr")
        nc.vector.tensor_reduce(z[:], ex[:][:, :, 0:kg], mybir.AxisListType.X,
                                OP.add)
        nc.vector.reciprocal(zr[:], z[:])
        for h in range(heads):
            nc.vector.tensor_tensor(
                gt[:][:, 0:kg, h * hc:(h + 1) * hc],
                gt[:][:, 0:kg, h * hc:(h + 1) * hc],
                ex[:][:, h:h + 1, 0:kg].rearrange("p o k -> p k o")
                    .broadcast_to([128, kg, hc]),
                OP.mult)
        o_sb = pool.tile([128, feat], F32, tag="o")
        nc.vector.tensor_reduce(
            o_sb[:], gt[:].rearrange("p k c -> p c k")[:, 0:feat, 0:kg],
            mybir.AxisListType.X, OP.add)
        for h in range(heads):
            nc.vector.tensor_scalar_mul(
                o_sb[:][:, h * hc:(h + 1) * hc],
                o_sb[:][:, h * hc:(h + 1) * hc], zr[:][:, h:h + 1])
        out_group_cb(g, o_sb)


def build_launch_b(nc, KA, KB, ncols, repeat=0):
    tab = nc.dram_tensor("tab1", [NROWS, TROW1], F16, kind="ExternalInput").ap()
    idx = nc.dram_tensor("idx1", [128, ncols], I16, kind="ExternalInput").ap()
    adP = nc.dram_tensor("adP1", [128, NG, HEADS], F32, kind="ExternalInput").ap()
    b1c = nc.dram_tensor("b1c", [H1F, 1], F32, kind="ExternalInput").ap()
    W2 = nc.dram_tensor("W2f", [H1F, OUT_C], F16, kind="ExternalInput").ap()
    Ms2 = nc.dram_tensor("Ms2", [OUT_C, 1], F16, kind="ExternalInput").ap()
    Md2 = nc.dram_tensor("Md2", [OUT_C, 1], F16, kind="ExternalInput").ap()
    idm = nc.dram_tensor("idm", [128, 128], F32, kind="ExternalInput").ap()
    h2t = nc.dram_tensor("h2t", [OUT_C, SHP], F16, kind="ExternalOutput").ap()
    as2t = nc.dram_tensor("as2t", [1, SHP], F32, kind="ExternalOutput").ap()
    ad2t = nc.dram_tensor("ad2t", [1, SHP], F32, kind="ExternalOutput").ap()

    with tile.TileContext(nc) as tc:
        with tc.tile_pool(name="st", bufs=1) as spool, \
             tc.tile_pool(name="gp", bufs=3) as gpool, \
             tc.tile_pool(name="ps", bufs=2, space="PSUM") as pps:
            idx_sb = spool.tile([128, ncols], I16)
            adP_sb = spool.tile([128, NG, HEADS], F32)
            b1_sb = spool.tile([H1F, 1], F32)
            w2_sb = spool.tile([H1F, OUT_C], F16)
            ms_sb = spool.tile([OUT_C, 1], F16)
            md_sb = spool.tile([OUT_C, 1], F16)
            id_sb = spool.tile([128, 128], F32)
            elu_sb = spool.tile([H1F, SHP], F16)
            h2_sb = spool.tile([OUT_C, SHP], F16)
            as2_sb = spool.tile([1, SHP], F32)
            ad2_sb = spool.tile([1, SHP], F32)
            nc.sync.dma_start(idx_sb[:], idx)
            nc.sync.dma_start(adP_sb[:], adP)
            nc.sync.dma_start(b1_sb[:], b1c)
            nc.sync.dma_start(w2_sb[:], W2)
            nc.sync.dma_start(ms_sb[:], Ms2)
            nc.sync.dma_start(md_sb[:], Md2)
            nc.sync.dma_start(id_sb[:], idm)

            rep = ExitStack()
            if repeat:
                rep.enter_context(tc.For_i(0, repeat, 1))

            def finish_group(g, o_sb):
                pt = pps.tile([128, 128], F32, tag="pt")
                nc.tensor.transpose(pt[:], o_sb[:], id_sb[:])
                v = gpool.tile([128, 128], F32, tag="v")
                nc.scalar.activation(v[:], pt[:], ACT.Identity, bias=b1_sb[:])
                _elu(nc, gpool, v, 128, elu_sb[:][:, g * 128:(g + 1) * 128])

            _edge_phase(nc, tc, gpool, pps, tab[0:WIN, :], tab[OFF_B:NROWS, :],
                        idx_sb, adP_sb, KA, KB, HEADS, H1F, TROW1, F16,
                        finish_group)

            for o, n in _chunks(SHP):
                p2 = pps.tile([OUT_C, 512], F32, tag="p2")
                nc.tensor.matmul(p2[:, :n], w2_sb[:], elu_sb[:][:, o:o + n])
                nc.vector.tensor_copy(h2_sb[:][:, o:o + n], p2[:, :n])
            for o, n in _chunks(SHP):
                pa = pps.tile([1, 512], F32, tag="pa2")
                nc.tensor.matmul(pa[:, :n], ms_sb[:], h2_sb[:][:, o:o + n])
                nc.vector.tensor_copy(as2_sb[:][:, o:o + n], pa[:, :n])
                pd = pps.tile([1, 512], F32, tag="pd2")
                nc.tensor.matmul(pd[:, :n], md_sb[:], h2_sb[:][:, o:o + n])
                nc.vector.tensor_copy(ad2_sb[:][:, o:o + n], pd[:, :n])
            nc.sync.dma_start(h2t, h2_sb[:])
            nc.sync.dma_start(as2t, as2_sb[:])
            nc.sync.dma_start(ad2t, ad2_sb[:])
            rep.close()
    nc.compile()
    return nc


def build_launch_c(nc, KA, KB, ncols, repeat=0):
    tab = nc.dram_tensor("tab2", [NROWS, TROW2], F32, kind="ExternalInput").ap()
    idx = nc.dram_tensor("idx2", [128, ncols], I16, kind="ExternalInput").ap()
    adP = nc.dram_tensor("adP2", [128, NG, 1], F32, kind="ExternalInput").ap()
    b2c = nc.dram_tensor("b2c", [OUT_C, 1], F32, kind="ExternalInput").ap()
    xres = nc.dram_tensor("xresP", [OUT_C, SHP], F32, kind="ExternalInput").ap()
    Wc1 = nc.dram_tensor("Wc1f", [OUT_C, 64], F16, kind="ExternalInput").ap()
    bc1 = nc.dram_tensor("bc1c", [64, 1], F32, kind="ExternalInput").ap()
    Wc2 = nc.dram_tensor("Wc2f", [64, 2], F16, kind="ExternalInput").ap()
    bc2 = nc.dram_tensor("bc2c", [2, 1], F32, kind="ExternalInput").ap()
    idm = nc.dram_tensor("idm", [128, 128], F32, kind="ExternalInput").ap()
    yt = nc.dram_tensor("yt", [2, SHP], F32, kind="ExternalOutput").ap()

    with tile.TileContext(nc) as tc:
        with tc.tile_pool(name="st", bufs=1) as spool, \
             tc.tile_pool(name="gp", bufs=3) as gpool, \
             tc.tile_pool(name="ps", bufs=2, space="PSUM") as pps:
            idx_sb = spool.tile([128, ncols], I16)
            adP_sb = spool.tile([128, NG, 1], F32)
            b2_sb = spool.tile([OUT_C, 1], F32)
            xr_sb = spool.tile([OUT_C, SHP], F32)
            w1_sb = spool.tile([OUT_C, 64], F16)
            b1_sb = spool.tile([64, 1], F32)
            w2_sb = spool.tile([64, 2], F16)
            b2c_sb = spool.tile([2, 1], F32)
            id_sb = spool.tile([128, 128], F32)
            y0_sb = spool.tile([OUT_C, SHP], F16)
            y1_sb = spool.tile([64, SHP], F16)
            y_sb = spool.tile([2, SHP], F32)
            nc.sync.dma_start(idx_sb[:], idx)
            nc.sync.dma_start(adP_sb[:], adP)
            nc.sync.dma_start(b2_sb[:], b2c)
            nc.sync.dma_start(xr_sb[:], xres)
            nc.sync.dma_start(w1_sb[:], Wc1)
            nc.sync.dma_start(b1_sb[:], bc1)
            nc.sync.dma_start(w2_sb[:], Wc2)
            nc.sync.dma_start(b2c_sb[:], bc2)
            nc.sync.dma_start(id_sb[:], idm)

            rep = ExitStack()
            if repeat:
                rep.enter_context(tc.For_i(0, repeat, 1))

            def finish_group(g, o_sb):
                pt = pps.tile([OUT_C, 128], F32, tag="pt")
                nc.tensor.transpose(pt[:], o_sb[:][:, 0:OUT_C], id_sb[:])
                v = gpool.tile([OUT_C, 128], F32, tag="v")
                nc.scalar.activation(v[:], pt[:], ACT.Identity, bias=b2_sb[:])
                e = gpool.tile([OUT_C, 128], F32, tag="e")
                _elu(nc, gpool, v, 128, e[:])
                nc.vector.tensor_tensor(
                    y0_sb[:][:, g * 128:(g + 1) * 128], e[:],
                    xr_sb[:][:, g * 128:(g + 1) * 128], OP.add)

            _edge_phase(nc, tc, gpool, pps, tab[0:WIN, :], tab[OFF_B:NROWS, :],
                        idx_sb, adP_sb, KA, KB, 1, OUT_C, TROW2, F32,
                        finish_group)

            for o, n in _chunks(SHP):
                p1 = pps.tile([64, 512], F32, tag="p1")
                nc.tensor.matmul(p1[:, :n], w1_sb[:], y0_sb[:][:, o:o + n])
                nc.scalar.activation(y1_sb[:][:, o:o + n], p1[:, :n], ACT.Relu,
                                     bias=b1_sb[:])
                p2 = pps.tile([2, 512], F32, tag="p2")
                nc.tensor.matmul(p2[:, :n], w2_sb[:], y1_sb[:][:, o:o + n])
                nc.scalar.activation(y_sb[:][:, o:o + n], p2[:, :n],
                                     ACT.Identity, bias=b2c_sb[:])
            nc.sync.dma_start(yt, y_sb[:])
            rep.close()
    nc.compile()
    return nc


# ------------------------------------------------------------------- kernel

_LAST_RUNS = []


def _run(nc, in_maps, name=""):
    _LAST_RUNS.append((name, nc, in_maps))
    return bass_utils.run_bass_kernel_spmd(nc, in_maps,
                                           core_ids=list(range(NCORES)))


def _ms_mat(a, heads, hid):
    m = np.zeros((heads * hid, heads), np.float32)
    for h in range(heads):
        m[h * hid:(h + 1) * hid, h] = a[h]
    return m


_CACHE = {}


def _get_programs(edge_index):
    key = edge_index.tobytes()[:64] + str(edge_index.sum()).encode()
    if key not in _CACHE:
        cores, KA, KB, idx_maps = _plan(edge_index)
        ncols = int(8 * (KA.sum() + KB.sum()))
        nca = build_launch_a(bacc.Bacc("TRN2", target_bir_lowering=False,
                                       debug=False, num_devices=NCORES))
        ncb = build_launch_b(bacc.Bacc("TRN2", target_bir_lowering=False,
                                       debug=False, num_devices=NCORES),
                             KA, KB, ncols)
        ncc = build_launch_c(bacc.Bacc("TRN2", target_bir_lowering=False,
                                       debug=False, num_devices=NCORES),
                             KA, KB, ncols)
        _CACHE[key] = (cores, KA, KB, idx_maps, ncols, nca, ncb, ncc)
    return _CACHE[key]


def kernel(x, edge_index, W1, a_src1, a_dst1, b1, W2, a_src2, a_dst2, b2,
           Wres, bres, Wc1, bc1, Wc2, bc2):
    x = np.asarray(x, np.float32)
    edge_index = np.asarray(edge_index, np.int32)
    cores, KA, KB, idx_maps, ncols, nca, ncb, ncc = _get_programs(edge_index)

    idm = np.eye(128, dtype=np.float32)

    # ---- launch A: node phase 1 (h1 = x@W1, alpha_s/d, residual)
    W1f = np.asarray(W1, np.float16)
    Ms1 = _ms_mat(np.asarray(a_src1), HEADS, HID).astype(np.float16)
    Md1 = _ms_mat(np.asarray(a_dst1), HEADS, HID).astype(np.float16)
    Wresf = np.asarray(Wres, np.float16)
    bresc = np.asarray(bres, np.float32).reshape(OUT_C, 1)
    in_a = []
    for c in range(NCORES):
        xT = np.ascontiguousarray(x[c * SH:(c + 1) * SH].T.astype(np.float16))
        in_a.append(dict(xT=xT, W1f=W1f, Ms1=Ms1, Md1=Md1, Wresf=Wresf,
                         bres=bresc))
    _LAST_RUNS.clear()
    res_a = _run(nca, in_a, 'A')

    h1 = np.concatenate([res_a.results[c]["h1t"].T for c in range(NCORES)], 0)
    as1 = np.concatenate([res_a.results[c]["as1t"].T for c in range(NCORES)], 0)
    ad1 = np.concatenate([res_a.results[c]["ad1t"].T for c in range(NCORES)], 0)
    xresT = [res_a.results[c]["xrest"] for c in range(NCORES)]

    # ---- host: pack gather table 1 (fp16, 512B rows) + permuted ad columns
    tab1 = np.zeros((NROWS, TROW1), np.float16)
    tab1[1:N + 1, :H1F] = h1.astype(np.float16)
    tab1[1:N + 1, H1F:H1F + HEADS] = as1.astype(np.float16)
    tab1[0, H1F:H1F + HEADS] = -60000.0
    tab1[N + 1, H1F:H1F + HEADS] = -60000.0

    b1c = np.asarray(b1, np.float32).reshape(H1F, 1)
    W2f = np.asarray(W2, np.float16)
    Ms2 = _ms_mat(np.asarray(a_src2), 1, OUT_C).astype(np.float16)
    Md2 = _ms_mat(np.asarray(a_dst2), 1, OUT_C).astype(np.float16)
    in_b = []
    for c in range(NCORES):
        perm = cores[c]["perm"]
        adp = np.zeros((SHP, HEADS), np.float32)
        adp[:SH] = ad1[c * SH + perm]
        adp = adp.reshape(NG, 128, HEADS).transpose(1, 0, 2)
        in_b.append(dict(tab1=tab1, idx1=idx_maps[c],
                         adP1=np.ascontiguousarray(adp), b1c=b1c, W2f=W2f,
                         Ms2=Ms2, Md2=Md2, idm=idm))
    res_b = _run(ncb, in_b, 'B')

    # ---- host: pack gather table 2 (fp32, 512B rows); un-permute h2/as2/ad2
    h2 = np.zeros((N, OUT_C), np.float32)
    as2 = np.zeros(N, np.float32)
    ad2 = np.zeros(N, np.float32)
    for c in range(NCORES):
        perm = cores[c]["perm"]
        h2[c * SH + perm] = res_b.results[c]["h2t"].T[:SH].astype(np.float32)
        as2[c * SH + perm] = res_b.results[c]["as2t"][0, :SH]
        ad2[c * SH + perm] = res_b.results[c]["ad2t"][0, :SH]
    tab2 = np.zeros((NROWS, TROW2), np.float32)
    tab2[1:N + 1, :OUT_C] = h2
    tab2[1:N + 1, OUT_C] = as2
    tab2[0, OUT_C] = -1e30
    tab2[N + 1, OUT_C] = -1e30

    b2c = np.asarray(b2, np.float32).reshape(OUT_C, 1)
    Wc1f = np.asarray(Wc1, np.float16)
    bc1c = np.asarray(bc1, np.float32).reshape(64, 1)
    Wc2f = np.asarray(Wc2, np.float16)
    bc2c = np.asarray(bc2, np.float32).reshape(2, 1)
    in_c = []
    for c in range(NCORES):
        perm = cores[c]["perm"]
        adp = np.zeros((SHP, 1), np.float32)
        adp[:SH, 0] = ad2[c * SH + perm]
        adp = adp.reshape(NG, 128, 1).transpose(1, 0, 2)
        xrp = np.zeros((OUT_C, SHP), np.float32)
        xrp[:, :SH] = xresT[c][:, perm]
        in_c.append(dict(tab2=tab2, idx2=idx_maps[c],
                         adP2=np.ascontiguousarray(adp), b2c=b2c,
                         xresP=xrp, Wc1f=Wc1f, bc1c=bc1c, Wc2f=Wc2f,
                         bc2c=bc2c, idm=idm))
    res_c = _run(ncc, in_c, 'C')

    out = np.zeros((N, 2), np.float32)
    for c in range(NCORES):
        perm = cores[c]["perm"]
        out[c * SH + perm] = res_c.results[c]["yt"].T[:SH]
    return out


# revision 9
# speedup vs baseline: 133.2228x; 1.3405x over previous
"""GAT (2-layer, residual, classifier) on 8 Trainium2 NeuronCores.

Strategy (graph/data parallel, per the sharding hint):
 - Nodes sharded by range across 8 cores; each core owns the edges whose
   destination falls in its range (segment softmax + aggregation are
   dst-local).
 - Node-phase matmuls run feature-major (features on partitions).
 - Edge phase uses a degree-bucketed ELL layout: per 128-node group, each
   node's incident edges occupy K slots along the free dimension; source-node
   feature rows (h | alpha_src packed into 512B rows) are fetched with
   dma_gather, softmax coefficients and the weighted aggregation run on the
   vector engine as per-partition ops, so no scatter is ever needed.
 - dma_gather indices are int16, so the 50002-row feature table is addressed
   through two overlapping 32768-row windows (A = rows 0..32767,
   B = rows 17234..50001); every edge is assigned a window and each group's
   slot columns are split into an A-run and a B-run. Padding slots point at
   dummy rows whose alpha_src is -60000 -> exp(logit) == 0, so they
   contribute nothing.
 - Three launches: A (node phase 1), B (edge phase 1 + node phase 2),
   C (edge phase 2 + residual + classifier). Between launches the host only
   reshapes/transposes/casts device-produced tensors into gather tables.
"""

import numpy as np
from contextlib import ExitStack

import concourse.bass as bass
import concourse.mybir as mybir
import concourse.tile as tile
import concourse.bacc as bacc
from concourse import bass_utils

# problem shape (hardcoded per contest contract)
N = 50000
E = 800000
IN_C = 128
HID = 32
HEADS = 4
H1F = HEADS * HID  # 128
OUT_C = 64
NEG = 0.2
NCORES = 8
SH = N // NCORES          # 6250 nodes per core
NG = (SH + 127) // 128    # 49 groups of 128 node-slots
SHP = NG * 128            # 6272 padded node slots

NROWS = N + 2             # dummyA, nodes, dummyB
WIN = 32768
OFF_B = NROWS - WIN       # 17234
A_MAX_SRC = WIN - 2       # 32766: last src reachable via window A (row=src+1)
B_MIN_SRC = OFF_B - 1     # 17233: first src reachable via window B
DUMMY_A = 0
DUMMY_B = WIN - 1         # 32767

TROW1 = 256               # fp16 elems per table-1 row (512B)
TROW2 = 128               # fp32 elems per table-2 row (512B)

F16 = mybir.dt.float16
F32 = mybir.dt.float32
I16 = mybir.dt.int16
OP = mybir.AluOpType
ACT = mybir.ActivationFunctionType


# ---------------------------------------------------------------- host plan

def _wrap_idx(val):
    """[128, K] int16 slot values -> dma_gather wrapped index layout
    [128, 8*K] (element i of the flat gather order at [i%16, i//16],
    replicated to 128 partitions)."""
    p, k = val.shape
    assert p == 128
    w = val.reshape(8, 16, k).transpose(1, 2, 0).reshape(16, 8 * k)
    return np.tile(w, (8, 1))


def _plan(edge_index):
    src = np.concatenate([edge_index[0], np.arange(N, dtype=np.int64)])
    dst = np.concatenate([edge_index[1], np.arange(N, dtype=np.int64)])
    cores = []
    for c in range(NCORES):
        lo = c * SH
        m = (dst >= lo) & (dst < lo + SH)
        s = src[m].astype(np.int64)
        d = (dst[m] - lo).astype(np.int64)
        o = np.argsort(d, kind="stable")
        s, d = s[o], d[o]
        deg = np.bincount(d, minlength=SH)
        amust = np.bincount(d[s <= B_MIN_SRC - 1], minlength=SH)
        bmust = np.bincount(d[s >= A_MAX_SRC + 1], minlength=SH)
        po = np.argsort(-deg, kind="stable")
        parts = []
        for i in range(0, SH, 512):
            w = po[i:i + 512]
            parts.append(w[np.argsort(-amust[w], kind="stable")])
        perm = np.concatenate(parts)
        starts = np.concatenate([[0], np.cumsum(deg)])
        cores.append(dict(s=s, deg=deg, a=amust, b=bmust, perm=perm, starts=starts))

    KA = np.zeros(NG, np.int64)
    KB = np.zeros(NG, np.int64)
    for g in range(NG):
        for p in cores:
            nodes = p["perm"][g * 128:(g + 1) * 128]
            if len(nodes):
                KA[g] = max(KA[g], p["a"][nodes].max())
    for g in range(NG):
        for p in cores:
            nodes = p["perm"][g * 128:(g + 1) * 128]
            if len(nodes):
                KB[g] = max(KB[g], p["b"][nodes].max(),
                            p["deg"][nodes].max() - KA[g])
    KA = np.maximum(KA, 1)
    KB = np.maximum(KB, 1)

    idx_maps = []
    for p in cores:
        blocks = []
        for g in range(NG):
            nodes = p["perm"][g * 128:(g + 1) * 128]
            vA = np.full((128, KA[g]), DUMMY_A, np.int16)
            vB = np.full((128, KB[g]), DUMMY_B, np.int16)
            for pi, n in enumerate(nodes):
                es = p["s"][p["starts"][n]:p["starts"][n + 1]]
                sa = es[es <= B_MIN_SRC - 1]
                sb = es[es >= A_MAX_SRC + 1]
                fl = es[(es > B_MIN_SRC - 1) & (es < A_MAX_SRC + 1)]
                a_load = max(len(sa), len(es) - KB[g])
                take = a_load - len(sa)
                av = np.concatenate([sa, fl[:take]]) + 1
                bv = np.concatenate([fl[take:], sb]) + 1 - OFF_B
                assert len(av) <= KA[g] and len(bv) <= KB[g]
                vA[pi, :len(av)] = av.astype(np.int16)
                vB[pi, :len(bv)] = bv.astype(np.int16)
            blocks.append(_wrap_idx(vA))
            blocks.append(_wrap_idx(vB))
        idx_maps.append(np.ascontiguousarray(np.concatenate(blocks, axis=1)))

    return cores, KA, KB, idx_maps


# ------------------------------------------------------------ launch builders

def _chunks(total, step=512):
    return [(o, min(step, total - o)) for o in range(0, total, step)]


def _elu(nc, pool, v, cols, out_ap):
    """out = elu(v) for an f-major fp32 SBUF tile v [P, cols]; out_ap may be
    a different dtype (cast on the final op)."""
    r = pool.tile(list(v.shape), F32, tag="elu_r")
    m = pool.tile(list(v.shape), F32, tag="elu_m")
    nc.vector.tensor_scalar_max(r[:], v[:], 0.0)
    nc.vector.tensor_scalar_min(m[:], v[:], 0.0)
    nc.scalar.activation(m[:], m[:], ACT.Exp)
    nc.vector.tensor_tensor(r[:], r[:], m[:], OP.add)
    nc.vector.tensor_scalar_add(out_ap, r[:], -1.0)


def build_launch_a(nc, repeat=0):
    xT = nc.dram_tensor("xT", [IN_C, SH], F16, kind="ExternalInput").ap()
    W1 = nc.dram_tensor("W1f", [IN_C, H1F], F16, kind="ExternalInput").ap()
    Ms1 = nc.dram_tensor("Ms1", [H1F, HEADS], F16, kind="ExternalInput").ap()
    Md1 = nc.dram_tensor("Md1", [H1F, HEADS], F16, kind="ExternalInput").ap()
    Wres = nc.dram_tensor("Wresf", [IN_C, OUT_C], F16, kind="ExternalInput").ap()
    bres = nc.dram_tensor("bres", [OUT_C, 1], F32, kind="ExternalInput").ap()
    h1t = nc.dram_tensor("h1t", [H1F, SH], F16, kind="ExternalOutput").ap()
    as1t = nc.dram_tensor("as1t", [HEADS, SH], F32, kind="ExternalOutput").ap()
    ad1t = nc.dram_tensor("ad1t", [HEADS, SH], F32, kind="ExternalOutput").ap()
    xrest = nc.dram_tensor("xrest", [OUT_C, SH], F32, kind="ExternalOutput").ap()

    with tile.TileContext(nc) as tc:
        with tc.tile_pool(name="sb", bufs=1) as pool, \
             tc.tile_pool(name="ps", bufs=2, space="PSUM") as pps:
            x_sb = pool.tile([IN_C, SH], F16)
            w1_sb = pool.tile([IN_C, H1F], F16)
            ms_sb = pool.tile([H1F, HEADS], F16)
            md_sb = pool.tile([H1F, HEADS], F16)
            wr_sb = pool.tile([IN_C, OUT_C], F16)
            br_sb = pool.tile([OUT_C, 1], F32)
            h1_sb = pool.tile([H1F, SH], F16)
            as_sb = pool.tile([HEADS, SH], F32)
            ad_sb = pool.tile([HEADS, SH], F32)
            xr_sb = pool.tile([OUT_C, SH], F32)
            nc.sync.dma_start(x_sb[:], xT)
            nc.sync.dma_start(w1_sb[:], W1)
            nc.sync.dma_start(ms_sb[:], Ms1)
            nc.sync.dma_start(md_sb[:], Md1)
            nc.sync.dma_start(wr_sb[:], Wres)
            nc.sync.dma_start(br_sb[:], bres)
            rep = ExitStack()
            if repeat:
                rep.enter_context(tc.For_i(0, repeat, 1))
            for o, n in _chunks(SH):
                ph = pps.tile([H1F, 512], F32, tag="ph")
                nc.tensor.matmul(ph[:, :n], w1_sb[:], x_sb[:, o:o + n])
                nc.vector.tensor_copy(h1_sb[:, o:o + n], ph[:, :n])
                pr = pps.tile([OUT_C, 512], F32, tag="pr")
                nc.tensor.matmul(pr[:, :n], wr_sb[:], x_sb[:, o:o + n])
                nc.scalar.activation(xr_sb[:, o:o + n], pr[:, :n], ACT.Identity,
                                     bias=br_sb[:])
            for o, n in _chunks(SH):
                pa = pps.tile([HEADS, 512], F32, tag="pa")
                nc.tensor.matmul(pa[:, :n], ms_sb[:], h1_sb[:, o:o + n])
                nc.vector.tensor_copy(as_sb[:, o:o + n], pa[:, :n])
                pd = pps.tile([HEADS, 512], F32, tag="pd")
                nc.tensor.matmul(pd[:, :n], md_sb[:], h1_sb[:, o:o + n])
                nc.vector.tensor_copy(ad_sb[:, o:o + n], pd[:, :n])
            nc.sync.dma_start(h1t, h1_sb[:])
            nc.sync.dma_start(as1t, as_sb[:])
            nc.sync.dma_start(ad1t, ad_sb[:])
            nc.sync.dma_start(xrest, xr_sb[:])
            rep.close()
    nc.compile()
    return nc


def _edge_phase(nc, tc, pool, pps, gwinA, gwinB, idx_sb, adP_sb, KA, KB,
                heads, feat, trow, gdt, out_group_cb):
    """Shared edge phase: for each group, gather + segment softmax + weighted
    aggregation; calls out_group_cb(g, o_sb) with the node-major [128, feat]
    fp32 aggregate."""
    kmax = int(max(KA[g] + KB[g] for g in range(NG)))
    hc = feat // heads
    icol = 0
    qn = 0
    for g in range(NG):
        ka, kb, kg = int(KA[g]), int(KB[g]), int(KA[g] + KB[g])
        gt = pool.tile([128, kmax, trow], gdt, tag="gt")
        nc.gpsimd.dma_gather(
            gt[:][:, 0:ka, :], gwinA, idx_sb[:][:, icol:icol + 8 * ka],
            128 * ka, 128 * ka, trow, single_packet=False, queue_num=qn)
        qn = (qn + 1) % 4
        icol += 8 * ka
        nc.gpsimd.dma_gather(
            gt[:][:, ka:kg, :], gwinB, idx_sb[:][:, icol:icol + 8 * kb],
            128 * kb, 128 * kb, trow, single_packet=False, queue_num=qn)
        qn = (qn + 1) % 4
        icol += 8 * kb

        lg = pool.tile([128, heads, kmax], F32, tag="lg")
        lt = pool.tile([128, heads, kmax], F32, tag="lt")
        ex = pool.tile([128, heads, kmax], F16, tag="ex")
        for h in range(heads):
            nc.vector.tensor_scalar(
                lg[:][:, h, 0:kg], gt[:][:, 0:kg, feat + h],
                adP_sb[:][:, g, h:h + 1], None, OP.add)
        # leaky_relu then exp
        nc.vector.tensor_scalar_mul(lt[:][:, :, 0:kg], lg[:][:, :, 0:kg], NEG)
        nc.vector.tensor_tensor(lg[:][:, :, 0:kg], lg[:][:, :, 0:kg],
                                lt[:][:, :, 0:kg], OP.max)
        nc.scalar.activation(ex[:][:, :, 0:kg], lg[:][:, :, 0:kg], ACT.Exp)

        z = pool.tile([128, heads], F32, tag="z")
        zr = pool.tile([128, heads], F32, tag="zr")
        nc.vector.tensor_reduce(z[:], ex[:][:, :, 0:kg], mybir.AxisListType.X,
                                OP.add)
        nc.vector.reciprocal(zr[:], z[:])
        for h in range(heads):
            nc.vector.tensor_tensor(
                gt[:][:, 0:kg, h * hc:(h + 1) * hc],
                gt[:][:, 0:kg, h * hc:(h + 1) * hc],
                ex[:][:, h:h + 1, 0:kg].rearrange("p o k -> p k o")
                    .broadcast_to([128, kg, hc]),
                OP.mult)
        o_sb = pool.tile([128, feat], F32, tag="o")
        nc.vector.tensor_reduce(
            o_sb[:], gt[:].rearrange("p k c -> p c k")[:, 0:feat, 0:kg],
            mybir.AxisListType.X, OP.add)
        for h in range(heads):
            nc.vector.tensor_scalar_mul(
                o_sb[:][:, h * hc:(h + 1) * hc],
                o_sb[:][:, h * hc:(h + 1) * hc], zr[:][:, h:h + 1])
        out_group_cb(g, o_sb)


def build_launch_b(nc, KA, KB, ncols, repeat=0):
    tab = nc.dram_tensor("tab1", [NROWS, TROW1], F16, kind="ExternalInput").ap()
    idx = nc.dram_tensor("idx1", [128, ncols], I16, kind="ExternalInput").ap()
    adP = nc.dram_tensor("adP1", [128, NG, HEADS], F32, kind="ExternalInput").ap()
    b1c = nc.dram_tensor("b1c", [H1F, 1], F32, kind="ExternalInput").ap()
    W2 = nc.dram_tensor("W2f", [H1F, OUT_C], F16, kind="ExternalInput").ap()
    Ms2 = nc.dram_tensor("Ms2", [OUT_C, 1], F16, kind="ExternalInput").ap()
    Md2 = nc.dram_tensor("Md2", [OUT_C, 1], F16, kind="ExternalInput").ap()
    idm = nc.dram_tensor("idm", [128, 128], F32, kind="ExternalInput").ap()
    h2t = nc.dram_tensor("h2t", [OUT_C, SHP], F16, kind="ExternalOutput").ap()
    as2t = nc.dram_tensor("as2t", [1, SHP], F32, kind="ExternalOutput").ap()
    ad2t = nc.dram_tensor("ad2t", [1, SHP], F32, kind="ExternalOutput").ap()

    with tile.TileContext(nc) as tc:
        with tc.tile_pool(name="st", bufs=1) as spool, \
             tc.tile_pool(name="gp", bufs=4) as gpool, \
             tc.tile_pool(name="ps", bufs=2, space="PSUM") as pps:
            idx_sb = spool.tile([128, ncols], I16)
            adP_sb = spool.tile([128, NG, HEADS], F32)
            b1_sb = spool.tile([H1F, 1], F32)
            w2_sb = spool.tile([H1F, OUT_C], F16)
            ms_sb = spool.tile([OUT_C, 1], F16)
            md_sb = spool.tile([OUT_C, 1], F16)
            id_sb = spool.tile([128, 128], F32)
            elu_sb = spool.tile([H1F, SHP], F16)
            h2_sb = spool.tile([OUT_C, SHP], F16)
            as2_sb = spool.tile([1, SHP], F32)
            ad2_sb = spool.tile([1, SHP], F32)
            nc.sync.dma_start(idx_sb[:], idx)
            nc.sync.dma_start(adP_sb[:], adP)
            nc.sync.dma_start(b1_sb[:], b1c)
            nc.sync.dma_start(w2_sb[:], W2)
            nc.sync.dma_start(ms_sb[:], Ms2)
            nc.sync.dma_start(md_sb[:], Md2)
            nc.sync.dma_start(id_sb[:], idm)

            rep = ExitStack()
            if repeat:
                rep.enter_context(tc.For_i(0, repeat, 1))

            def finish_group(g, o_sb):
                pt = pps.tile([128, 128], F32, tag="pt")
                nc.tensor.transpose(pt[:], o_sb[:], id_sb[:])
                v = gpool.tile([128, 128], F32, tag="v")
                nc.scalar.activation(v[:], pt[:], ACT.Identity, bias=b1_sb[:])
                _elu(nc, gpool, v, 128, elu_sb[:][:, g * 128:(g + 1) * 128])

            _edge_phase(nc, tc, gpool, pps, tab[0:WIN, :], tab[OFF_B:NROWS, :],
                        idx_sb, adP_sb, KA, KB, HEADS, H1F, TROW1, F16,
                        finish_group)

            for o, n in _chunks(SHP):
                p2 = pps.tile([OUT_C, 512], F32, tag="p2")
                nc.tensor.matmul(p2[:, :n], w2_sb[:], elu_sb[:][:, o:o + n])
                nc.vector.tensor_copy(h2_sb[:][:, o:o + n], p2[:, :n])
            for o, n in _chunks(SHP):
                pa = pps.tile([1, 512], F32, tag="pa2")
                nc.tensor.matmul(pa[:, :n], ms_sb[:], h2_sb[:][:, o:o + n])
                nc.vector.tensor_copy(as2_sb[:][:, o:o + n], pa[:, :n])
                pd = pps.tile([1, 512], F32, tag="pd2")
                nc.tensor.matmul(pd[:, :n], md_sb[:], h2_sb[:][:, o:o + n])
                nc.vector.tensor_copy(ad2_sb[:][:, o:o + n], pd[:, :n])
            nc.sync.dma_start(h2t, h2_sb[:])
            nc.sync.dma_start(as2t, as2_sb[:])
            nc.sync.dma_start(ad2t, ad2_sb[:])
            rep.close()
    nc.compile()
    return nc


def build_launch_c(nc, KA, KB, ncols, repeat=0):
    tab = nc.dram_tensor("tab2", [NROWS, TROW2], F32, kind="ExternalInput").ap()
    idx = nc.dram_tensor("idx2", [128, ncols], I16, kind="ExternalInput").ap()
    adP = nc.dram_tensor("adP2", [128, NG, 1], F32, kind="ExternalInput").ap()
    b2c = nc.dram_tensor("b2c", [OUT_C, 1], F32, kind="ExternalInput").ap()
    xres = nc.dram_tensor("xresP", [OUT_C, SHP], F32, kind="ExternalInput").ap()
    Wc1 = nc.dram_tensor("Wc1f", [OUT_C, 64], F16, kind="ExternalInput").ap()
    bc1 = nc.dram_tensor("bc1c", [64, 1], F32, kind="ExternalInput").ap()
    Wc2 = nc.dram_tensor("Wc2f", [64, 2], F16, kind="ExternalInput").ap()
    bc2 = nc.dram_tensor("bc2c", [2, 1], F32, kind="ExternalInput").ap()
    idm = nc.dram_tensor("idm", [128, 128], F32, kind="ExternalInput").ap()
    yt = nc.dram_tensor("yt", [2, SHP], F32, kind="ExternalOutput").ap()

    with tile.TileContext(nc) as tc:
        with tc.tile_pool(name="st", bufs=1) as spool, \
             tc.tile_pool(name="gp", bufs=4) as gpool, \
             tc.tile_pool(name="ps", bufs=2, space="PSUM") as pps:
            idx_sb = spool.tile([128, ncols], I16)
            adP_sb = spool.tile([128, NG, 1], F32)
            b2_sb = spool.tile([OUT_C, 1], F32)
            xr_sb = spool.tile([OUT_C, SHP], F32)
            w1_sb = spool.tile([OUT_C, 64], F16)
            b1_sb = spool.tile([64, 1], F32)
            w2_sb = spool.tile([64, 2], F16)
            b2c_sb = spool.tile([2, 1], F32)
            id_sb = spool.tile([128, 128], F32)
            y0_sb = spool.tile([OUT_C, SHP], F16)
            y1_sb = spool.tile([64, SHP], F16)
            y_sb = spool.tile([2, SHP], F32)
            nc.sync.dma_start(idx_sb[:], idx)
            nc.sync.dma_start(adP_sb[:], adP)
            nc.sync.dma_start(b2_sb[:], b2c)
            nc.sync.dma_start(xr_sb[:], xres)
            nc.sync.dma_start(w1_sb[:], Wc1)
            nc.sync.dma_start(b1_sb[:], bc1)
            nc.sync.dma_start(w2_sb[:], Wc2)
            nc.sync.dma_start(b2c_sb[:], bc2)
            nc.sync.dma_start(id_sb[:], idm)

            rep = ExitStack()
            if repeat:
                rep.enter_context(tc.For_i(0, repeat, 1))

            def finish_group(g, o_sb):
                pt = pps.tile([OUT_C, 128], F32, tag="pt")
                nc.tensor.transpose(pt[:], o_sb[:][:, 0:OUT_C], id_sb[:])
                v = gpool.tile([OUT_C, 128], F32, tag="v")
                nc.scalar.activation(v[:], pt[:], ACT.Identity, bias=b2_sb[:])
                e = gpool.tile([OUT_C, 128], F32, tag="e")
                _elu(nc, gpool, v, 128, e[:])
                nc.vector.tensor_tensor(
                    y0_sb[:][:, g * 128:(g + 1) * 128], e[:],
                    xr_sb[:][:, g * 128:(g + 1) * 128], OP.add)

            _edge_phase(nc, tc, gpool, pps, tab[0:WIN, :], tab[OFF_B:NROWS, :],
                        idx_sb, adP_sb, KA, KB, 1, OUT_C, TROW2, F32,
                        finish_group)

            for o, n in _chunks(SHP):
                p1 = pps.tile([64, 512], F32, tag="p1")
                nc.tensor.matmul(p1[:, :n], w1_sb[:], y0_sb[:][:, o:o + n])
                nc.scalar.activation(y1_sb[:][:, o:o + n], p1[:, :n], ACT.Relu,
                                     bias=b1_sb[:])
                p2 = pps.tile([2, 512], F32, tag="p2")
                nc.tensor.matmul(p2[:, :n], w2_sb[:], y1_sb[:][:, o:o + n])
                nc.scalar.activation(y_sb[:][:, o:o + n], p2[:, :n],
                                     ACT.Identity, bias=b2c_sb[:])
            nc.sync.dma_start(yt, y_sb[:])
            rep.close()
    nc.compile()
    return nc


# ------------------------------------------------------------------- kernel

_LAST_RUNS = []


def _run(nc, in_maps, name=""):
    _LAST_RUNS.append((name, nc, in_maps))
    return bass_utils.run_bass_kernel_spmd(nc, in_maps,
                                           core_ids=list(range(NCORES)))


def _ms_mat(a, heads, hid):
    m = np.zeros((heads * hid, heads), np.float32)
    for h in range(heads):
        m[h * hid:(h + 1) * hid, h] = a[h]
    return m


_CACHE = {}


def _get_programs(edge_index):
    key = edge_index.tobytes()[:64] + str(edge_index.sum()).encode()
    if key not in _CACHE:
        cores, KA, KB, idx_maps = _plan(edge_index)
        ncols = int(8 * (KA.sum() + KB.sum()))
        nca = build_launch_a(bacc.Bacc("TRN2", target_bir_lowering=False,
                                       debug=False, num_devices=NCORES))
        ncb = build_launch_b(bacc.Bacc("TRN2", target_bir_lowering=False,
                                       debug=False, num_devices=NCORES,
                                       num_swdge_queues=4),
                             KA, KB, ncols)
        ncc = build_launch_c(bacc.Bacc("TRN2", target_bir_lowering=False,
                                       debug=False, num_devices=NCORES,
                                       num_swdge_queues=4),
                             KA, KB, ncols)
        _CACHE[key] = (cores, KA, KB, idx_maps, ncols, nca, ncb, ncc)
    return _CACHE[key]


def kernel(x, edge_index, W1, a_src1, a_dst1, b1, W2, a_src2, a_dst2, b2,
           Wres, bres, Wc1, bc1, Wc2, bc2):
    x = np.asarray(x, np.float32)
    edge_index = np.asarray(edge_index, np.int32)
    cores, KA, KB, idx_maps, ncols, nca, ncb, ncc = _get_programs(edge_index)

    idm = np.eye(128, dtype=np.float32)

    # ---- launch A: node phase 1 (h1 = x@W1, alpha_s/d, residual)
    W1f = np.asarray(W1, np.float16)
    Ms1 = _ms_mat(np.asarray(a_src1), HEADS, HID).astype(np.float16)
    Md1 = _ms_mat(np.asarray(a_dst1), HEADS, HID).astype(np.float16)
    Wresf = np.asarray(Wres, np.float16)
    bresc = np.asarray(bres, np.float32).reshape(OUT_C, 1)
    in_a = []
    for c in range(NCORES):
        xT = np.ascontiguousarray(x[c * SH:(c + 1) * SH].T.astype(np.float16))
        in_a.append(dict(xT=xT, W1f=W1f, Ms1=Ms1, Md1=Md1, Wresf=Wresf,
                         bres=bresc))
    _LAST_RUNS.clear()
    res_a = _run(nca, in_a, 'A')

    h1 = np.concatenate([res_a.results[c]["h1t"].T for c in range(NCORES)], 0)
    as1 = np.concatenate([res_a.results[c]["as1t"].T for c in range(NCORES)], 0)
    ad1 = np.concatenate([res_a.results[c]["ad1t"].T for c in range(NCORES)], 0)
    xresT = [res_a.results[c]["xrest"] for c in range(NCORES)]

    # ---- host: pack gather table 1 (fp16, 512B rows) + permuted ad columns
    tab1 = np.zeros((NROWS, TROW1), np.float16)
    tab1[1:N + 1, :H1F] = h1.astype(np.float16)
    tab1[1:N + 1, H1F:H1F + HEADS] = as1.astype(np.float16)
    tab1[0, H1F:H1F + HEADS] = -60000.0
    tab1[N + 1, H1F:H1F + HEADS] = -60000.0

    b1c = np.asarray(b1, np.float32).reshape(H1F, 1)
    W2f = np.asarray(W2, np.float16)
    Ms2 = _ms_mat(np.asarray(a_src2), 1, OUT_C).astype(np.float16)
    Md2 = _ms_mat(np.asarray(a_dst2), 1, OUT_C).astype(np.float16)
    in_b = []
    for c in range(NCORES):
        perm = cores[c]["perm"]
        adp = np.zeros((SHP, HEADS), np.float32)
        adp[:SH] = ad1[c * SH + perm]
        adp = adp.reshape(NG, 128, HEADS).transpose(1, 0, 2)
        in_b.append(dict(tab1=tab1, idx1=idx_maps[c],
                         adP1=np.ascontiguousarray(adp), b1c=b1c, W2f=W2f,
                         Ms2=Ms2, Md2=Md2, idm=idm))
    res_b = _run(ncb, in_b, 'B')

    # ---- host: pack gather table 2 (fp32, 512B rows); un-permute h2/as2/ad2
    h2 = np.zeros((N, OUT_C), np.float32)
    as2 = np.zeros(N, np.float32)
    ad2 = np.zeros(N, np.float32)
    for c in range(NCORES):
        perm = cores[c]["perm"]
        h2[c * SH + perm] = res_b.results[c]["h2t"].T[:SH].astype(np.float32)
        as2[c * SH + perm] = res_b.results[c]["as2t"][0, :SH]
        ad2[c * SH + perm] = res_b.results[c]["ad2t"][0, :SH]
    tab2 = np.zeros((NROWS, TROW2), np.float32)
    tab2[1:N + 1, :OUT_C] = h2
    tab2[1:N + 1, OUT_C] = as2
    tab2[0, OUT_C] = -1e30
    tab2[N + 1, OUT_C] = -1e30

    b2c = np.asarray(b2, np.float32).reshape(OUT_C, 1)
    Wc1f = np.asarray(Wc1, np.float16)
    bc1c = np.asarray(bc1, np.float32).reshape(64, 1)
    Wc2f = np.asarray(Wc2, np.float16)
    bc2c = np.asarray(bc2, np.float32).reshape(2, 1)
    in_c = []
    for c in range(NCORES):
        perm = cores[c]["perm"]
        adp = np.zeros((SHP, 1), np.float32)
        adp[:SH, 0] = ad2[c * SH + perm]
        adp = adp.reshape(NG, 128, 1).transpose(1, 0, 2)
        xrp = np.zeros((OUT_C, SHP), np.float32)
        xrp[:, :SH] = xresT[c][:, perm]
        in_c.append(dict(tab2=tab2, idx2=idx_maps[c],
                         adP2=np.ascontiguousarray(adp), b2c=b2c,
                         xresP=xrp, Wc1f=Wc1f, bc1c=bc1c, Wc2f=Wc2f,
                         bc2c=bc2c, idm=idm))
    res_c = _run(ncc, in_c, 'C')

    out = np.zeros((N, 2), np.float32)
    for c in range(NCORES):
        perm = cores[c]["perm"]
        out[c * SH + perm] = res_c.results[c]["yt"].T[:SH]
    return out


# revision 10
# speedup vs baseline: 154.3167x; 1.1583x over previous
"""GAT (2-layer, residual, classifier) on 8 Trainium2 NeuronCores.

Strategy (graph/data parallel, per the sharding hint):
 - Nodes sharded by range across 8 cores; each core owns the edges whose
   destination falls in its range (segment softmax + aggregation are
   dst-local).
 - Node-phase matmuls run feature-major (features on partitions).
 - Edge phase uses a degree-bucketed ELL layout: per 128-node group, each
   node's incident edges occupy K slots along the free dimension; source-node
   feature rows (h | alpha_src packed into 512B rows) are fetched with
   dma_gather, softmax coefficients and the weighted aggregation run on the
   vector engine as per-partition ops, so no scatter is ever needed.
 - dma_gather indices are int16, so the 50002-row feature table is addressed
   through two overlapping 32768-row windows (A = rows 0..32767,
   B = rows 17234..50001); every edge is assigned a window and each group's
   slot columns are split into an A-run and a B-run. Padding slots point at
   dummy rows whose alpha_src is -60000 -> exp(logit) == 0, so they
   contribute nothing.
 - Three launches: A (node phase 1), B (edge phase 1 + node phase 2),
   C (edge phase 2 + residual + classifier). Between launches the host only
   reshapes/transposes/casts device-produced tensors into gather tables.
"""

import numpy as np
from contextlib import ExitStack

import concourse.bass as bass
import concourse.mybir as mybir
import concourse.tile as tile
import concourse.bacc as bacc
from concourse import bass_utils

# problem shape (hardcoded per contest contract)
N = 50000
E = 800000
IN_C = 128
HID = 32
HEADS = 4
H1F = HEADS * HID  # 128
OUT_C = 64
NEG = 0.2
NCORES = 8
SH = N // NCORES          # 6250 nodes per core
NG = (SH + 127) // 128    # 49 groups of 128 node-slots
SHP = NG * 128            # 6272 padded node slots

NROWS = N + 2             # dummyA, nodes, dummyB
WIN = 32768
OFF_B = NROWS - WIN       # 17234
A_MAX_SRC = WIN - 2       # 32766: last src reachable via window A (row=src+1)
B_MIN_SRC = OFF_B - 1     # 17233: first src reachable via window B
DUMMY_A = 0
DUMMY_B = WIN - 1         # 32767

TROW1 = 256               # fp16 elems per table-1 row (512B)
TROW2 = 128               # fp32 elems per table-2 row (512B)

F16 = mybir.dt.float16
F32 = mybir.dt.float32
I16 = mybir.dt.int16
OP = mybir.AluOpType
ACT = mybir.ActivationFunctionType


# ---------------------------------------------------------------- host plan

def _wrap_idx(val):
    """[128, K] int16 slot values -> dma_gather wrapped index layout
    [128, 8*K] (element i of the flat gather order at [i%16, i//16],
    replicated to 128 partitions)."""
    p, k = val.shape
    assert p == 128
    w = val.reshape(8, 16, k).transpose(1, 2, 0).reshape(16, 8 * k)
    return np.tile(w, (8, 1))


def _plan(edge_index):
    src = np.concatenate([edge_index[0], np.arange(N, dtype=np.int64)])
    dst = np.concatenate([edge_index[1], np.arange(N, dtype=np.int64)])
    cores = []
    for c in range(NCORES):
        lo = c * SH
        m = (dst >= lo) & (dst < lo + SH)
        s = src[m].astype(np.int64)
        d = (dst[m] - lo).astype(np.int64)
        o = np.argsort(d, kind="stable")
        s, d = s[o], d[o]
        deg = np.bincount(d, minlength=SH)
        amust = np.bincount(d[s <= B_MIN_SRC - 1], minlength=SH)
        bmust = np.bincount(d[s >= A_MAX_SRC + 1], minlength=SH)
        po = np.argsort(-deg, kind="stable")
        parts = []
        for i in range(0, SH, 512):
            w = po[i:i + 512]
            parts.append(w[np.argsort(-amust[w], kind="stable")])
        perm = np.concatenate(parts)
        starts = np.concatenate([[0], np.cumsum(deg)])
        cores.append(dict(s=s, deg=deg, a=amust, b=bmust, perm=perm, starts=starts))

    KA = np.zeros(NG, np.int64)
    KB = np.zeros(NG, np.int64)
    for g in range(NG):
        for p in cores:
            nodes = p["perm"][g * 128:(g + 1) * 128]
            if len(nodes):
                KA[g] = max(KA[g], p["a"][nodes].max())
    for g in range(NG):
        for p in cores:
            nodes = p["perm"][g * 128:(g + 1) * 128]
            if len(nodes):
                KB[g] = max(KB[g], p["b"][nodes].max(),
                            p["deg"][nodes].max() - KA[g])
    KA = np.maximum(KA, 1)
    KB = np.maximum(KB, 1)

    idx_maps = []
    for p in cores:
        blocks = []
        for g in range(NG):
            nodes = p["perm"][g * 128:(g + 1) * 128]
            vA = np.full((128, KA[g]), DUMMY_A, np.int16)
            vB = np.full((128, KB[g]), DUMMY_B, np.int16)
            for pi, n in enumerate(nodes):
                es = p["s"][p["starts"][n]:p["starts"][n + 1]]
                sa = es[es <= B_MIN_SRC - 1]
                sb = es[es >= A_MAX_SRC + 1]
                fl = es[(es > B_MIN_SRC - 1) & (es < A_MAX_SRC + 1)]
                a_load = max(len(sa), len(es) - KB[g])
                take = a_load - len(sa)
                av = np.concatenate([sa, fl[:take]]) + 1
                bv = np.concatenate([fl[take:], sb]) + 1 - OFF_B
                assert len(av) <= KA[g] and len(bv) <= KB[g]
                vA[pi, :len(av)] = av.astype(np.int16)
                vB[pi, :len(bv)] = bv.astype(np.int16)
            blocks.append(_wrap_idx(vA))
            blocks.append(_wrap_idx(vB))
        idx_maps.append(np.ascontiguousarray(np.concatenate(blocks, axis=1)))

    return cores, KA, KB, idx_maps


# ------------------------------------------------------------ launch builders

def _chunks(total, step=512):
    return [(o, min(step, total - o)) for o in range(0, total, step)]


def _elu(nc, pool, v, cols, out_ap):
    """out = elu(v) for an f-major fp32 SBUF tile v [P, cols]; out_ap may be
    a different dtype (cast on the final op)."""
    r = pool.tile(list(v.shape), F32, tag="elu_r")
    m = pool.tile(list(v.shape), F32, tag="elu_m")
    nc.vector.tensor_scalar_max(r[:], v[:], 0.0)
    nc.vector.tensor_scalar_min(m[:], v[:], 0.0)
    nc.scalar.activation(m[:], m[:], ACT.Exp)
    nc.vector.tensor_tensor(r[:], r[:], m[:], OP.add)
    nc.vector.tensor_scalar_add(out_ap, r[:], -1.0)


def build_launch_a(nc, repeat=0):
    xT = nc.dram_tensor("xT", [IN_C, SH], F16, kind="ExternalInput").ap()
    W1 = nc.dram_tensor("W1f", [IN_C, H1F], F16, kind="ExternalInput").ap()
    Ms1 = nc.dram_tensor("Ms1", [H1F, HEADS], F16, kind="ExternalInput").ap()
    Md1 = nc.dram_tensor("Md1", [H1F, HEADS], F16, kind="ExternalInput").ap()
    Wres = nc.dram_tensor("Wresf", [IN_C, OUT_C], F16, kind="ExternalInput").ap()
    bres = nc.dram_tensor("bres", [OUT_C, 1], F32, kind="ExternalInput").ap()
    h1t = nc.dram_tensor("h1t", [H1F, SH], F16, kind="ExternalOutput").ap()
    as1t = nc.dram_tensor("as1t", [HEADS, SH], F32, kind="ExternalOutput").ap()
    ad1t = nc.dram_tensor("ad1t", [HEADS, SH], F32, kind="ExternalOutput").ap()
    xrest = nc.dram_tensor("xrest", [OUT_C, SH], F32, kind="ExternalOutput").ap()

    with tile.TileContext(nc) as tc:
        with tc.tile_pool(name="sb", bufs=1) as pool, \
             tc.tile_pool(name="ps", bufs=2, space="PSUM") as pps:
            x_sb = pool.tile([IN_C, SH], F16)
            w1_sb = pool.tile([IN_C, H1F], F16)
            ms_sb = pool.tile([H1F, HEADS], F16)
            md_sb = pool.tile([H1F, HEADS], F16)
            wr_sb = pool.tile([IN_C, OUT_C], F16)
            br_sb = pool.tile([OUT_C, 1], F32)
            h1_sb = pool.tile([H1F, SH], F16)
            as_sb = pool.tile([HEADS, SH], F32)
            ad_sb = pool.tile([HEADS, SH], F32)
            xr_sb = pool.tile([OUT_C, SH], F32)
            nc.sync.dma_start(x_sb[:], xT)
            nc.sync.dma_start(w1_sb[:], W1)
            nc.sync.dma_start(ms_sb[:], Ms1)
            nc.sync.dma_start(md_sb[:], Md1)
            nc.sync.dma_start(wr_sb[:], Wres)
            nc.sync.dma_start(br_sb[:], bres)
            rep = ExitStack()
            if repeat:
                rep.enter_context(tc.For_i(0, repeat, 1))
            for o, n in _chunks(SH):
                ph = pps.tile([H1F, 512], F32, tag="ph")
                nc.tensor.matmul(ph[:, :n], w1_sb[:], x_sb[:, o:o + n])
                nc.vector.tensor_copy(h1_sb[:, o:o + n], ph[:, :n])
                pr = pps.tile([OUT_C, 512], F32, tag="pr")
                nc.tensor.matmul(pr[:, :n], wr_sb[:], x_sb[:, o:o + n])
                nc.scalar.activation(xr_sb[:, o:o + n], pr[:, :n], ACT.Identity,
                                     bias=br_sb[:])
            for o, n in _chunks(SH):
                pa = pps.tile([HEADS, 512], F32, tag="pa")
                nc.tensor.matmul(pa[:, :n], ms_sb[:], h1_sb[:, o:o + n])
                nc.vector.tensor_copy(as_sb[:, o:o + n], pa[:, :n])
                pd = pps.tile([HEADS, 512], F32, tag="pd")
                nc.tensor.matmul(pd[:, :n], md_sb[:], h1_sb[:, o:o + n])
                nc.vector.tensor_copy(ad_sb[:, o:o + n], pd[:, :n])
            nc.sync.dma_start(h1t, h1_sb[:])
            nc.sync.dma_start(as1t, as_sb[:])
            nc.sync.dma_start(ad1t, ad_sb[:])
            nc.sync.dma_start(xrest, xr_sb[:])
            rep.close()
    nc.compile()
    return nc


def _edge_phase(nc, tc, pool, pps, gwinA, gwinB, idx_sb, adP_sb, KA, KB,
                heads, feat, trow, gdt, out_group_cb):
    """Shared edge phase: for each group, gather + segment softmax + weighted
    aggregation; calls out_group_cb(g, o_sb) with the node-major [128, feat]
    fp32 aggregate."""
    kmax = int(max(KA[g] + KB[g] for g in range(NG)))
    hc = feat // heads
    icol = 0
    qn = 0
    for g in range(NG):
        ka, kb, kg = int(KA[g]), int(KB[g]), int(KA[g] + KB[g])
        gt = pool.tile([128, kmax, trow], gdt, tag="gt")
        nc.gpsimd.dma_gather(
            gt[:][:, 0:ka, :], gwinA, idx_sb[:][:, icol:icol + 8 * ka],
            128 * ka, 128 * ka, trow, single_packet=False, queue_num=qn)
        qn = (qn + 1) % 4
        icol += 8 * ka
        nc.gpsimd.dma_gather(
            gt[:][:, ka:kg, :], gwinB, idx_sb[:][:, icol:icol + 8 * kb],
            128 * kb, 128 * kb, trow, single_packet=False, queue_num=qn)
        qn = (qn + 1) % 4
        icol += 8 * kb

        lg = pool.tile([128, heads, kmax], F32, tag="lg")
        lt = pool.tile([128, heads, kmax], F32, tag="lt")
        ex = pool.tile([128, heads, kmax], F16, tag="ex")
        nc.vector.tensor_tensor(
            lg[:][:, :, 0:kg],
            gt[:][:, 0:kg, feat:feat + heads].rearrange("p k h -> p h k"),
            adP_sb[:][:, g, :].unsqueeze(2).broadcast_to([128, heads, kg]),
            OP.add)
        # leaky_relu then exp
        nc.vector.tensor_scalar_mul(lt[:][:, :, 0:kg], lg[:][:, :, 0:kg], NEG)
        nc.vector.tensor_tensor(lg[:][:, :, 0:kg], lg[:][:, :, 0:kg],
                                lt[:][:, :, 0:kg], OP.max)
        nc.scalar.activation(ex[:][:, :, 0:kg], lg[:][:, :, 0:kg], ACT.Exp)

        z = pool.tile([128, heads], F32, tag="z")
        zr = pool.tile([128, heads], F32, tag="zr")
        nc.vector.tensor_reduce(z[:], ex[:][:, :, 0:kg], mybir.AxisListType.X,
                                OP.add)
        nc.vector.reciprocal(zr[:], z[:])
        g4 = gt[:][:, 0:kg, 0:feat].rearrange("p k (h c) -> p k h c", h=heads)
        e4 = (ex[:][:, :, 0:kg].rearrange("p h k -> p k h").unsqueeze(3)
              .broadcast_to([128, kg, heads, hc]))
        nc.vector.tensor_tensor(g4, g4, e4, OP.mult)
        o_sb = pool.tile([128, feat], F32, tag="o")
        nc.vector.tensor_reduce(
            o_sb[:], gt[:].rearrange("p k c -> p c k")[:, 0:feat, 0:kg],
            mybir.AxisListType.X, OP.add)
        for h in range(heads):
            nc.vector.tensor_scalar_mul(
                o_sb[:][:, h * hc:(h + 1) * hc],
                o_sb[:][:, h * hc:(h + 1) * hc], zr[:][:, h:h + 1])
        out_group_cb(g, o_sb)


def build_launch_b(nc, KA, KB, ncols, repeat=0):
    tab = nc.dram_tensor("tab1", [NROWS, TROW1], F16, kind="ExternalInput").ap()
    idx = nc.dram_tensor("idx1", [128, ncols], I16, kind="ExternalInput").ap()
    adP = nc.dram_tensor("adP1", [128, NG, HEADS], F32, kind="ExternalInput").ap()
    b1c = nc.dram_tensor("b1c", [H1F, 1], F32, kind="ExternalInput").ap()
    W2 = nc.dram_tensor("W2f", [H1F, OUT_C], F16, kind="ExternalInput").ap()
    Ms2 = nc.dram_tensor("Ms2", [OUT_C, 1], F16, kind="ExternalInput").ap()
    Md2 = nc.dram_tensor("Md2", [OUT_C, 1], F16, kind="ExternalInput").ap()
    idm = nc.dram_tensor("idm", [128, 128], F32, kind="ExternalInput").ap()
    h2t = nc.dram_tensor("h2t", [OUT_C, SHP], F16, kind="ExternalOutput").ap()
    as2t = nc.dram_tensor("as2t", [1, SHP], F32, kind="ExternalOutput").ap()
    ad2t = nc.dram_tensor("ad2t", [1, SHP], F32, kind="ExternalOutput").ap()

    with tile.TileContext(nc) as tc:
        with tc.tile_pool(name="st", bufs=1) as spool, \
             tc.tile_pool(name="gp", bufs=4) as gpool, \
             tc.tile_pool(name="ps", bufs=2, space="PSUM") as pps:
            idx_sb = spool.tile([128, ncols], I16)
            adP_sb = spool.tile([128, NG, HEADS], F32)
            b1_sb = spool.tile([H1F, 1], F32)
            w2_sb = spool.tile([H1F, OUT_C], F16)
            ms_sb = spool.tile([OUT_C, 1], F16)
            md_sb = spool.tile([OUT_C, 1], F16)
            id_sb = spool.tile([128, 128], F32)
            elu_sb = spool.tile([H1F, SHP], F16)
            h2_sb = spool.tile([OUT_C, SHP], F16)
            as2_sb = spool.tile([1, SHP], F32)
            ad2_sb = spool.tile([1, SHP], F32)
            nc.sync.dma_start(idx_sb[:], idx)
            nc.sync.dma_start(adP_sb[:], adP)
            nc.sync.dma_start(b1_sb[:], b1c)
            nc.sync.dma_start(w2_sb[:], W2)
            nc.sync.dma_start(ms_sb[:], Ms2)
            nc.sync.dma_start(md_sb[:], Md2)
            nc.sync.dma_start(id_sb[:], idm)

            rep = ExitStack()
            if repeat:
                rep.enter_context(tc.For_i(0, repeat, 1))

            def finish_group(g, o_sb):
                pt = pps.tile([128, 128], F32, tag="pt")
                nc.tensor.transpose(pt[:], o_sb[:], id_sb[:])
                v = gpool.tile([128, 128], F32, tag="v")
                nc.scalar.activation(v[:], pt[:], ACT.Identity, bias=b1_sb[:])
                _elu(nc, gpool, v, 128, elu_sb[:][:, g * 128:(g + 1) * 128])

            _edge_phase(nc, tc, gpool, pps, tab[0:WIN, :], tab[OFF_B:NROWS, :],
                        idx_sb, adP_sb, KA, KB, HEADS, H1F, TROW1, F16,
                        finish_group)

            for o, n in _chunks(SHP):
                p2 = pps.tile([OUT_C, 512], F32, tag="p2")
                nc.tensor.matmul(p2[:, :n], w2_sb[:], elu_sb[:][:, o:o + n])
                nc.vector.tensor_copy(h2_sb[:][:, o:o + n], p2[:, :n])
            for o, n in _chunks(SHP):
                pa = pps.tile([1, 512], F32, tag="pa2")
                nc.tensor.matmul(pa[:, :n], ms_sb[:], h2_sb[:][:, o:o + n])
                nc.vector.tensor_copy(as2_sb[:][:, o:o + n], pa[:, :n])
                pd = pps.tile([1, 512], F32, tag="pd2")
                nc.tensor.matmul(pd[:, :n], md_sb[:], h2_sb[:][:, o:o + n])
                nc.vector.tensor_copy(ad2_sb[:][:, o:o + n], pd[:, :n])
            nc.sync.dma_start(h2t, h2_sb[:])
            nc.sync.dma_start(as2t, as2_sb[:])
            nc.sync.dma_start(ad2t, ad2_sb[:])
            rep.close()
    nc.compile()
    return nc


def build_launch_c(nc, KA, KB, ncols, repeat=0):
    tab = nc.dram_tensor("tab2", [NROWS, TROW2], F32, kind="ExternalInput").ap()
    idx = nc.dram_tensor("idx2", [128, ncols], I16, kind="ExternalInput").ap()
    adP = nc.dram_tensor("adP2", [128, NG, 1], F32, kind="ExternalInput").ap()
    b2c = nc.dram_tensor("b2c", [OUT_C, 1], F32, kind="ExternalInput").ap()
    xres = nc.dram_tensor("xresP", [OUT_C, SHP], F32, kind="ExternalInput").ap()
    Wc1 = nc.dram_tensor("Wc1f", [OUT_C, 64], F16, kind="ExternalInput").ap()
    bc1 = nc.dram_tensor("bc1c", [64, 1], F32, kind="ExternalInput").ap()
    Wc2 = nc.dram_tensor("Wc2f", [64, 2], F16, kind="ExternalInput").ap()
    bc2 = nc.dram_tensor("bc2c", [2, 1], F32, kind="ExternalInput").ap()
    idm = nc.dram_tensor("idm", [128, 128], F32, kind="ExternalInput").ap()
    yt = nc.dram_tensor("yt", [2, SHP], F32, kind="ExternalOutput").ap()

    with tile.TileContext(nc) as tc:
        with tc.tile_pool(name="st", bufs=1) as spool, \
             tc.tile_pool(name="gp", bufs=4) as gpool, \
             tc.tile_pool(name="ps", bufs=2, space="PSUM") as pps:
            idx_sb = spool.tile([128, ncols], I16)
            adP_sb = spool.tile([128, NG, 1], F32)
            b2_sb = spool.tile([OUT_C, 1], F32)
            xr_sb = spool.tile([OUT_C, SHP], F32)
            w1_sb = spool.tile([OUT_C, 64], F16)
            b1_sb = spool.tile([64, 1], F32)
            w2_sb = spool.tile([64, 2], F16)
            b2c_sb = spool.tile([2, 1], F32)
            id_sb = spool.tile([128, 128], F32)
            y0_sb = spool.tile([OUT_C, SHP], F16)
            y1_sb = spool.tile([64, SHP], F16)
            y_sb = spool.tile([2, SHP], F32)
            nc.sync.dma_start(idx_sb[:], idx)
            nc.sync.dma_start(adP_sb[:], adP)
            nc.sync.dma_start(b2_sb[:], b2c)
            nc.sync.dma_start(xr_sb[:], xres)
            nc.sync.dma_start(w1_sb[:], Wc1)
            nc.sync.dma_start(b1_sb[:], bc1)
            nc.sync.dma_start(w2_sb[:], Wc2)
            nc.sync.dma_start(b2c_sb[:], bc2)
            nc.sync.dma_start(id_sb[:], idm)

            rep = ExitStack()
            if repeat:
                rep.enter_context(tc.For_i(0, repeat, 1))

            def finish_group(g, o_sb):
                pt = pps.tile([OUT_C, 128], F32, tag="pt")
                nc.tensor.transpose(pt[:], o_sb[:][:, 0:OUT_C], id_sb[:])
                v = gpool.tile([OUT_C, 128], F32, tag="v")
                nc.scalar.activation(v[:], pt[:], ACT.Identity, bias=b2_sb[:])
                e = gpool.tile([OUT_C, 128], F32, tag="e")
                _elu(nc, gpool, v, 128, e[:])
                nc.vector.tensor_tensor(
                    y0_sb[:][:, g * 128:(g + 1) * 128], e[:],
                    xr_sb[:][:, g * 128:(g + 1) * 128], OP.add)

            _edge_phase(nc, tc, gpool, pps, tab[0:WIN, :], tab[OFF_B:NROWS, :],
                        idx_sb, adP_sb, KA, KB, 1, OUT_C, TROW2, F32,
                        finish_group)

            for o, n in _chunks(SHP):
                p1 = pps.tile([64, 512], F32, tag="p1")
                nc.tensor.matmul(p1[:, :n], w1_sb[:], y0_sb[:][:, o:o + n])
                nc.scalar.activation(y1_sb[:][:, o:o + n], p1[:, :n], ACT.Relu,
                                     bias=b1_sb[:])
                p2 = pps.tile([2, 512], F32, tag="p2")
                nc.tensor.matmul(p2[:, :n], w2_sb[:], y1_sb[:][:, o:o + n])
                nc.scalar.activation(y_sb[:][:, o:o + n], p2[:, :n],
                                     ACT.Identity, bias=b2c_sb[:])
            nc.sync.dma_start(yt, y_sb[:])
            rep.close()
    nc.compile()
    return nc


# ------------------------------------------------------------------- kernel

_LAST_RUNS = []


def _run(nc, in_maps, name=""):
    _LAST_RUNS.append((name, nc, in_maps))
    return bass_utils.run_bass_kernel_spmd(nc, in_maps,
                                           core_ids=list(range(NCORES)))


def _ms_mat(a, heads, hid):
    m = np.zeros((heads * hid, heads), np.float32)
    for h in range(heads):
        m[h * hid:(h + 1) * hid, h] = a[h]
    return m


_CACHE = {}


def _get_programs(edge_index):
    key = edge_index.tobytes()[:64] + str(edge_index.sum()).encode()
    if key not in _CACHE:
        cores, KA, KB, idx_maps = _plan(edge_index)
        ncols = int(8 * (KA.sum() + KB.sum()))
        nca = build_launch_a(bacc.Bacc("TRN2", target_bir_lowering=False,
                                       debug=False, num_devices=NCORES))
        ncb = build_launch_b(bacc.Bacc("TRN2", target_bir_lowering=False,
                                       debug=False, num_devices=NCORES,
                                       num_swdge_queues=4),
                             KA, KB, ncols)
        ncc = build_launch_c(bacc.Bacc("TRN2", target_bir_lowering=False,
                                       debug=False, num_devices=NCORES,
                                       num_swdge_queues=4),
                             KA, KB, ncols)
        _CACHE[key] = (cores, KA, KB, idx_maps, ncols, nca, ncb, ncc)
    return _CACHE[key]


def kernel(x, edge_index, W1, a_src1, a_dst1, b1, W2, a_src2, a_dst2, b2,
           Wres, bres, Wc1, bc1, Wc2, bc2):
    x = np.asarray(x, np.float32)
    edge_index = np.asarray(edge_index, np.int32)
    cores, KA, KB, idx_maps, ncols, nca, ncb, ncc = _get_programs(edge_index)

    idm = np.eye(128, dtype=np.float32)

    # ---- launch A: node phase 1 (h1 = x@W1, alpha_s/d, residual)
    W1f = np.asarray(W1, np.float16)
    Ms1 = _ms_mat(np.asarray(a_src1), HEADS, HID).astype(np.float16)
    Md1 = _ms_mat(np.asarray(a_dst1), HEADS, HID).astype(np.float16)
    Wresf = np.asarray(Wres, np.float16)
    bresc = np.asarray(bres, np.float32).reshape(OUT_C, 1)
    in_a = []
    for c in range(NCORES):
        xT = np.ascontiguousarray(x[c * SH:(c + 1) * SH].T.astype(np.float16))
        in_a.append(dict(xT=xT, W1f=W1f, Ms1=Ms1, Md1=Md1, Wresf=Wresf,
                         bres=bresc))
    _LAST_RUNS.clear()
    res_a = _run(nca, in_a, 'A')

    h1 = np.concatenate([res_a.results[c]["h1t"].T for c in range(NCORES)], 0)
    as1 = np.concatenate([res_a.results[c]["as1t"].T for c in range(NCORES)], 0)
    ad1 = np.concatenate([res_a.results[c]["ad1t"].T for c in range(NCORES)], 0)
    xresT = [res_a.results[c]["xrest"] for c in range(NCORES)]

    # ---- host: pack gather table 1 (fp16, 512B rows) + permuted ad columns
    tab1 = np.zeros((NROWS, TROW1), np.float16)
    tab1[1:N + 1, :H1F] = h1.astype(np.float16)
    tab1[1:N + 1, H1F:H1F + HEADS] = as1.astype(np.float16)
    tab1[0, H1F:H1F + HEADS] = -60000.0
    tab1[N + 1, H1F:H1F + HEADS] = -60000.0

    b1c = np.asarray(b1, np.float32).reshape(H1F, 1)
    W2f = np.asarray(W2, np.float16)
    Ms2 = _ms_mat(np.asarray(a_src2), 1, OUT_C).astype(np.float16)
    Md2 = _ms_mat(np.asarray(a_dst2), 1, OUT_C).astype(np.float16)
    in_b = []
    for c in range(NCORES):
        perm = cores[c]["perm"]
        adp = np.zeros((SHP, HEADS), np.float32)
        adp[:SH] = ad1[c * SH + perm]
        adp = adp.reshape(NG, 128, HEADS).transpose(1, 0, 2)
        in_b.append(dict(tab1=tab1, idx1=idx_maps[c],
                         adP1=np.ascontiguousarray(adp), b1c=b1c, W2f=W2f,
                         Ms2=Ms2, Md2=Md2, idm=idm))
    res_b = _run(ncb, in_b, 'B')

    # ---- host: pack gather table 2 (fp32, 512B rows); un-permute h2/as2/ad2
    h2 = np.zeros((N, OUT_C), np.float32)
    as2 = np.zeros(N, np.float32)
    ad2 = np.zeros(N, np.float32)
    for c in range(NCORES):
        perm = cores[c]["perm"]
        h2[c * SH + perm] = res_b.results[c]["h2t"].T[:SH].astype(np.float32)
        as2[c * SH + perm] = res_b.results[c]["as2t"][0, :SH]
        ad2[c * SH + perm] = res_b.results[c]["ad2t"][0, :SH]
    tab2 = np.zeros((NROWS, TROW2), np.float32)
    tab2[1:N + 1, :OUT_C] = h2
    tab2[1:N + 1, OUT_C] = as2
    tab2[0, OUT_C] = -1e30
    tab2[N + 1, OUT_C] = -1e30

    b2c = np.asarray(b2, np.float32).reshape(OUT_C, 1)
    Wc1f = np.asarray(Wc1, np.float16)
    bc1c = np.asarray(bc1, np.float32).reshape(64, 1)
    Wc2f = np.asarray(Wc2, np.float16)
    bc2c = np.asarray(bc2, np.float32).reshape(2, 1)
    in_c = []
    for c in range(NCORES):
        perm = cores[c]["perm"]
        adp = np.zeros((SHP, 1), np.float32)
        adp[:SH, 0] = ad2[c * SH + perm]
        adp = adp.reshape(NG, 128, 1).transpose(1, 0, 2)
        xrp = np.zeros((OUT_C, SHP), np.float32)
        xrp[:, :SH] = xresT[c][:, perm]
        in_c.append(dict(tab2=tab2, idx2=idx_maps[c],
                         adP2=np.ascontiguousarray(adp), b2c=b2c,
                         xresP=xrp, Wc1f=Wc1f, bc1c=bc1c, Wc2f=Wc2f,
                         bc2c=bc2c, idm=idm))
    res_c = _run(ncc, in_c, 'C')

    out = np.zeros((N, 2), np.float32)
    for c in range(NCORES):
        perm = cores[c]["perm"]
        out[c * SH + perm] = res_c.results[c]["yt"].T[:SH]
    return out
